# revision 22
# baseline (speedup 1.0000x reference)
"""MoE (top-2 of 8 experts, SwiGLU FFN) on 8 Trainium2 NeuronCores.

Strategy: expert-parallel with H-sliced load balancing. The gate/top-k
routing runs on host (bit-exact with the reference: jax on CPU). The 8
experts are split into 2 groups of 4 (rank-matched by token count);
each group owns 4 cores, and each core computes ONE H-quarter (768 of
3072) of ALL 4 experts in its group. Per-core tensor-engine work is
then (c1+c3+c5+c7)/4 token-FFN-equivalents (counts sorted desc) instead
of max(count) - within ~1% of the perfectly-balanced floor. The host
sums the four quarter-H partial outputs (fp32) and scatter-adds into
the full [B,S,D] output.

Problem dims (hardcoded): B=4, S=2048, D=1024, E=8, TOP_K=2, H=3072.

Perf notes (from NTFF traces):
- The PE matmul stream runs at the bf16 roofline (~N/2.4GHz per MM)
  once data is resident; an early version lost ~26us waiting for the
  gpsimd/SWDGE queue (boots at ~12us, ~76GB/s) to deliver chunk-0
  tokens, while the sync HWDGE queue moved weights at ~400GB/s.
- ALL input DMA rides the sync HWDGE queue, ordered by first use:
  phase-0 chunk-0 tokens (one 3D-AP dma_start - per-k slices measured
  WORSE due to ~0.62us/dispatch serialization), w13 in per-ht blocks
  (host-prepacked so each block is one contiguous 4KB-per-partition
  transfer), wts, next tokens, w2, then later phases' weights. Stage A
  consumes w13 ht-blocks as they arrive (3.4us compute per block vs
  ~1.4us DMA) so the PE starts ~14us in and never starves.
- Output (yg) DMA uses the scalar HWDGE queue so it never head-of-line
  blocks input prefetches on the sync queue; one contiguous [128,1024]
  store per token-tile. fp32 output: bf16 measured slightly WORSE.
- Chunks are >=231 wide: at small N the 128-column LDWEIGHTS stream
  (~107ns @1.2GHz) outpaces the matmul stream and the PE gets
  LDW-paced. Token counts are exact (matmul free dims are arbitrary);
  only chunk STARTS must be 128-aligned for wts indexing.

SBUF budget per partition (bytes): w13 4x24K=96K, w2 4x12K=48K,
xg 16K, g 6K, sig+sil 6K, ot 12K -> ~184K of 208K.
PSUM: ps1 x2 + ps3 x2 (stage A) + pso x4 (stage B) = 8 banks.
"""

import sys
import types

if "/opt/trn_rl_repo" not in sys.path:
    sys.path.insert(0, "/opt/trn_rl_repo")

import numpy as np
import ml_dtypes


def _ensure_axon_hooks_shim():
    """bass_utils imports antenv.axon_hooks when BASS_TRACE is set; this
    image's antenv lacks it. Provide a no-op shim so tracing degrades
    gracefully instead of crashing (a real hook may overwrite it)."""
    try:
        import antenv.axon_hooks  # noqa: F401
        return
    except ImportError:
        pass
    try:
        import antenv
    except ImportError:
        return
    mod = types.ModuleType("antenv.axon_hooks")
    mod._hook = None
    mod.set_axon_ntff_profile_hook = lambda h: setattr(mod, "_hook", h)
    mod.get_axon_ntff_profile_hook = lambda: mod._hook
    sys.modules["antenv.axon_hooks"] = mod
    antenv.axon_hooks = mod


_ensure_axon_hooks_shim()

B, S, D = 4, 2048, 1024
E = 8
TOP_K = 2
H = 3 * D
T = B * S
KD = D // 128       # 8  k-tiles over D
M = 4               # experts per core = H-split factor
NG = E // M         # 2  groups (each group: M experts on M cores)
SH = H // M         # 768: H-slice per core
NSH = SH // 128     # 6  h-tiles per slice
ND = D // 512       # 2  512-wide output column tiles
BLK = 2 * KD * 128  # free-dim extent of one w13 ht-block (w1|w3 x 8 k-tiles)

BF16 = ml_dtypes.bfloat16

_nc_cache: dict = {}


def _chunk_list(C):
    """Token chunks (PSUM bank free dim <= 512). All chunk STARTS are
    128-aligned (wts indexing) and all chunks are >=231 wide. Only the
    last chunk may be a non-multiple of 128. Chunk 0 must be 512 so
    phase-0 stage A outpaces the w13 block DMA."""
    chunks = []
    rem = C
    while rem > 512 + 384:
        chunks.append(512)
        rem -= 512
    if rem <= 512:
        chunks.append(rem)
    else:
        # split so the first part is a multiple of 128 and both >= 231
        first = min(512, ((rem - 231) // 128) * 128)
        chunks.extend([first, rem - first])
    return chunks


def build_multi_ffn(caps):
    """Bass program for one core: SwiGLU FFN over one H-slice (H/M) of
    M experts; phase i processes caps[i] tokens.

    Inputs (all host-prepacked, per core), i = 0..M-1:
      xgk{i}  [128, KD*CP_i]   bf16 : xgk[p, k*CP+c] = x[token c, k*128+p]
      w13b{i} [128, NSH*BLK]   bf16 : per-ht blocks of this core's H-slice:
                                      w13b[p, ((ht*2+s)*KD+k)*128+c]
                                        = w{1,3}[k*128+p, (q*NSH+ht)*128+c]
      w2{i}   [SH, D]          bf16 : this core's H-slice rows of w2
      wts{i}  [128, CP_i/128]  f32  : combine weight of token n*128+p at [p,n]
    Outputs:
      yg{i} [CP_i, D]          f32  : partial (H-slice) expert outputs,
                                      wts * (silu(xg@w1s) * (xg@w3s)) @ w2s
    """
    import concourse.bacc as bacc
    import concourse.tile as tile
    import concourse.mybir as mybir

    fp32 = mybir.dt.float32
    bf16 = mybir.dt.bfloat16

    nc = bacc.Bacc("TRN2", target_bir_lowering=False, debug=False, num_devices=8)

    phases = []
    for i, C in enumerate(caps):
        CP = ((C + 127) // 128) * 128   # host arrays padded to 128 tokens
        phases.append({
            "C": C,
            "CP": CP,
            "chunks": _chunk_list(C),
            "xgk": nc.dram_tensor(f"xgk{i}", [128, KD * CP], bf16, kind="ExternalInput"),
            "w13b": nc.dram_tensor(f"w13b{i}", [128, NSH * BLK], bf16, kind="ExternalInput"),
            "w2": nc.dram_tensor(f"w2{i}", [SH, D], bf16, kind="ExternalInput"),
            "wts": nc.dram_tensor(f"wts{i}", [128, CP // 128], fp32, kind="ExternalInput"),
            "yg": nc.dram_tensor(f"yg{i}", [CP, D], fp32, kind="ExternalOutput"),
            "prefetched": {},
            "xg0": None,
        })

    with tile.TileContext(nc) as tc:
        with (
            tc.tile_pool(name="wres", bufs=1) as wres,
            tc.tile_pool(name="xgp", bufs=2) as xgp,
            tc.tile_pool(name="gp", bufs=1) as gp,
            tc.tile_pool(name="tmp", bufs=3) as tmp,
            tc.tile_pool(name="outp", bufs=3) as outp,
            tc.tile_pool(name="psA", bufs=2, space="PSUM") as psA,
            tc.tile_pool(name="psB", bufs=4, space="PSUM") as psB,
        ):
            def load_xg_chunk(ph, c0, NC):
                # one dma_start per chunk ([128, KD, NC] 3D AP): a single
                # ~0.62us sync-engine dispatch; splitting into per-k DMAs
                # was measured WORSE (8 serialized dispatches delay w13b0)
                xgk_3d = ph["xgk"].ap().rearrange("p (k c) -> p k c", k=KD)
                xt = xgp.tile([128, KD * NC], bf16, tag="xg")
                nc.sync.dma_start(
                    xt[:].rearrange("p (k c) -> p k c", k=KD),
                    xgk_3d[:, :, c0:c0 + NC],
                )
                return [xt[:, k * NC:(k + 1) * NC] for k in range(KD)]

            def load_weights(i, ph):
                w13_sb = []
                for ht in range(NSH):
                    t1 = wres.tile([128, BLK], bf16, tag=f"w13_{i}_{ht}")
                    nc.sync.dma_start(
                        t1[:], ph["w13b"].ap()[:, ht * BLK:(ht + 1) * BLK]
                    )
                    w13_sb.append(t1)
                wts_sb = wres.tile([128, ph["CP"] // 128], fp32, tag=f"wts{i}")
                nc.sync.dma_start(wts_sb[:], ph["wts"].ap())
                return w13_sb, wts_sb

            def load_w2(i, ph):
                w2_sb = []
                for ht in range(NSH):
                    t2 = wres.tile([128, D], bf16, tag=f"w2_{i}_{ht}")
                    nc.sync.dma_start(
                        t2[:], ph["w2"].ap()[ht * 128:(ht + 1) * 128, :]
                    )
                    w2_sb.append(t2)
                return w2_sb

            # Sync HWDGE queue FIFO, in consumption order: phase-0 chunk-0
            # tokens, phase-0 w13 blocks + wts, chunk-1 tokens, phase-0 w2,
            # then later phases' weights (all land by ~100us; phase 1 starts
            # ~135us). Remaining token chunks are prefetched from the loop.
            p0 = phases[0]
            p0["xg0"] = load_xg_chunk(p0, 0, p0["chunks"][0])
            p0["w13_sb"], p0["wts_sb"] = load_weights(0, p0)
            offs0 = [sum(p0["chunks"][:i]) for i in range(len(p0["chunks"]))]
            if len(p0["chunks"]) > 1:
                p0["prefetched"][1] = load_xg_chunk(p0, offs0[1], p0["chunks"][1])
            p0["w2_sb"] = load_w2(0, p0)
            for i in range(1, M):
                ph = phases[i]
                ph["w13_sb"], ph["wts_sb"] = load_weights(i, ph)
                ph["w2_sb"] = load_w2(i, ph)

            for pi, ph in enumerate(phases):
                chunks = ph["chunks"]
                offs = [sum(chunks[:i]) for i in range(len(chunks))]
                w13_sb, w2_sb, wts_sb = ph["w13_sb"], ph["w2_sb"], ph["wts_sb"]
                yg = ph["yg"]
                prefetched = ph["prefetched"]
                c0 = 0
                for ch, NC in enumerate(chunks):
                    NT = (NC + 127) // 128
                    xg_sb = ph["xg0"] if ch == 0 else prefetched.pop(ch)
                    if xg_sb is None:   # later phases: prefetched below
                        xg_sb = prefetched.pop(0)
                    # prefetch one chunk ahead; at this phase's last chunk,
                    # prefetch the next phase's chunk 0
                    if ch + 1 < len(chunks):
                        if ch + 1 not in prefetched:
                            prefetched[ch + 1] = load_xg_chunk(
                                ph, offs[ch + 1], chunks[ch + 1]
                            )
                    elif pi + 1 < len(phases):
                        nxt = phases[pi + 1]
                        nxt["prefetched"][0] = load_xg_chunk(
                            nxt, 0, nxt["chunks"][0]
                        )

                    # stage A: g[h, tok] = silu(y1) * y3 for this H-slice
                    g_tiles = []
                    for ht in range(NSH):
                        # phase-0 chunk-0: stage B is idle, so borrow psB's
                        # banks for extra in-flight groups while the w13
                        # blocks are still arriving
                        pool = psB if (pi == 0 and ch == 0 and ht % 2 == 1) else psA
                        ps1 = pool.tile([128, NC], fp32, tag="ps1" if pool is psA else "pso")
                        ps3 = pool.tile([128, NC], fp32, tag="ps3" if pool is psA else "pso")
                        wt = w13_sb[ht]
                        for k in range(KD):
                            nc.tensor.matmul(
                                ps1[:],
                                wt[:, k * 128:(k + 1) * 128],
                                xg_sb[k],
                                start=(k == 0),
                                stop=(k == KD - 1),
                            )
                        for k in range(KD):
                            nc.tensor.matmul(
                                ps3[:],
                                wt[:, (KD + k) * 128:(KD + k + 1) * 128],
                                xg_sb[k],
                                start=(k == 0),
                                stop=(k == KD - 1),
                            )
                        sig = tmp.tile([128, NC], fp32, tag="sig")
                        nc.scalar.activation(
                            sig[:], ps1[:], mybir.ActivationFunctionType.Sigmoid
                        )
                        sil = tmp.tile([128, NC], fp32, tag="sil")
                        nc.vector.tensor_mul(sil[:], sig[:], ps1[:])
                        gt = gp.tile([128, NC], bf16, tag=f"g_{ht}")
                        nc.vector.tensor_mul(gt[:], sil[:], ps3[:])
                        g_tiles.append(gt)

                    # stage B: yg[tok, d] = wts[tok] * (g.T @ w2slice)
                    for tt in range(NT):
                        gtile_idx = c0 // 128 + tt
                        pt = min(128, NC - tt * 128)   # exact tail width
                        ot = outp.tile([128, D], fp32, tag="ot")
                        for dh in range(ND):
                            pso = psB.tile([128, 512], fp32, tag="pso")
                            for ht in range(NSH):
                                nc.tensor.matmul(
                                    pso[:pt, :],
                                    g_tiles[ht][:, tt * 128:tt * 128 + pt],
                                    w2_sb[ht][:, dh * 512:(dh + 1) * 512],
                                    start=(ht == 0),
                                    stop=(ht == NSH - 1),
                                )
                            nc.vector.tensor_scalar_mul(
                                ot[:pt, dh * 512:(dh + 1) * 512], pso[:pt, :],
                                wts_sb[:pt, gtile_idx:gtile_idx + 1],
                            )
                        # one contiguous [pt, 1024] store per token-tile on
                        # the scalar HWDGE queue so it never head-of-line
                        # blocks input prefetches on sync
                        nc.scalar.dma_start(
                            yg.ap()[c0 + tt * 128:c0 + tt * 128 + pt, :],
                            ot[:pt, :],
                        )
                    c0 += NC

    nc.compile()
    return nc


def route_host(xf: np.ndarray, gate_w: np.ndarray):
    """Top-2 routing, bit-exact with the reference (jax on CPU)."""
    import jax
    import jax.numpy as jnp

    cpu = jax.devices("cpu")[0]
    with jax.default_device(cpu):
        xj = jax.device_put(xf, cpu)
        gj = jax.device_put(gate_w, cpu)
        probs = jax.nn.softmax(xj @ gj, axis=-1)
        vals, idx = jax.lax.top_k(probs, TOP_K)
        w = vals / jnp.sum(vals, axis=-1, keepdims=True)
    return np.asarray(idx), np.asarray(w)


def prepare_dispatch(x, gate_w):
    """Host routing + per-expert gather lists + rank-matched grouping.

    Group g (g=0..NG-1) gets experts order[i*NG+g] for i=0..M-1 and runs
    on cores g*M..g*M+M-1 (core g*M+q computes H-slice q). Phase i's
    capacity caps[i] = counts[order[i*NG]] covers both groups' phase-i
    experts, minimizing sum(caps) over all rank-matched groupings.
    """
    xf = np.ascontiguousarray(np.asarray(x).reshape(T, D), dtype=np.float32)
    gate_w = np.asarray(gate_w, dtype=np.float32)
    idx, w = route_host(xf, gate_w)
    tok_flat = np.repeat(np.arange(T), TOP_K)
    idx_flat = idx.ravel()
    w_flat = w.astype(np.float32).ravel()
    toks = []
    wts_list = []
    for e in range(E):
        sel = idx_flat == e
        toks.append(tok_flat[sel])
        wts_list.append(w_flat[sel])
    counts = np.array([len(t) for t in toks])
    order = np.argsort(-counts, kind="stable")
    groups = [[int(order[i * NG + g]) for i in range(M)] for g in range(NG)]
    caps = tuple(max(256, int(counts[order[i * NG]])) for i in range(M))
    return xf, toks, wts_list, groups, caps


def _pack_tokens(xf_bf, toks_e, C):
    """xgk [128, KD*CP]: xgk[p, k*CP+c] = x[token c, k*128+p] (bf16)."""
    CP = ((C + 127) // 128) * 128
    xgT = np.zeros((D, CP), dtype=BF16)
    xgT[:, :len(toks_e)] = xf_bf[toks_e].T
    return np.ascontiguousarray(
        xgT.reshape(KD, 128, CP).transpose(1, 0, 2).reshape(128, -1)
    )


def _pack_wts(wts_e, C):
    CP = ((C + 127) // 128) * 128
    wflat = np.zeros(CP, dtype=np.float32)
    wflat[:len(wts_e)] = wts_e
    return np.ascontiguousarray(wflat.reshape(CP // 128, 128).T)


def _pack_w13_slice(w1_e, w3_e, q):
    """Per-ht blocks of H-slice q: [128, NSH*BLK], block ht is
    [w1 k-tiles | w3 k-tiles], each k-tile 128 cols contiguous."""
    sl = slice(q * SH, (q + 1) * SH)
    w1b = np.asarray(w1_e[:, sl], dtype=np.float32).astype(BF16)
    w3b = np.asarray(w3_e[:, sl], dtype=np.float32).astype(BF16)
    # [D, SH] -> [k, p, ht, c] -> [p, ht, (s), k, c]
    w1r = w1b.reshape(KD, 128, NSH, 128).transpose(1, 2, 0, 3)
    w3r = w3b.reshape(KD, 128, NSH, 128).transpose(1, 2, 0, 3)
    return np.ascontiguousarray(np.stack([w1r, w3r], axis=2).reshape(128, -1))


def make_in_maps(xf, toks, wts_list, groups, caps, w1, w2, w3):
    xf_bf = xf.astype(BF16)
    in_maps = [dict() for _ in range(E)]
    for g in range(NG):
        for i in range(M):
            e = groups[g][i]
            xgk = _pack_tokens(xf_bf, toks[e], caps[i])
            wts = _pack_wts(wts_list[e], caps[i])
            for q in range(M):
                im = in_maps[g * M + q]
                im[f"xgk{i}"] = xgk
                im[f"wts{i}"] = wts
                im[f"w13b{i}"] = _pack_w13_slice(w1[e], w3[e], q)
                im[f"w2{i}"] = np.ascontiguousarray(
                    np.asarray(w2[e][q * SH:(q + 1) * SH], dtype=np.float32)
                ).astype(BF16)
    return in_maps


def combine_outputs(results, toks, groups):
    out = np.zeros((T, D), dtype=np.float32)
    for g in range(NG):
        for i in range(M):
            e = groups[g][i]
            n_e = len(toks[e])
            acc = np.array(results[g * M][f"yg{i}"][:n_e], dtype=np.float32)
            for q in range(1, M):
                acc += np.asarray(
                    results[g * M + q][f"yg{i}"][:n_e], dtype=np.float32
                )
            out[toks[e]] += acc
    return out.reshape(B, S, D)


def run(x, gate_w, w1, w2, w3, **spmd_kwargs):
    """Run the MoE. Returns (output, BassKernelResults)."""
    from concourse import bass_utils

    xf, toks, wts_list, groups, caps = prepare_dispatch(x, gate_w)
    if caps not in _nc_cache:
        _nc_cache[caps] = build_multi_ffn(caps)
    nc = _nc_cache[caps]

    in_maps = make_in_maps(xf, toks, wts_list, groups, caps, w1, w2, w3)
    res = bass_utils.run_bass_kernel_spmd(
        nc, in_maps, core_ids=list(range(E)), **spmd_kwargs
    )
    out = combine_outputs(res.results, toks, groups).astype(
        np.asarray(x).dtype, copy=False
    )
    return out, res


def kernel(x, gate_w, w1, w2, w3):
    out, _ = run(x, gate_w, w1, w2, w3)
    return out


# revision 23
# speedup vs baseline: 1.0191x; 1.0191x over previous
"""MoE (top-2 of 8 experts, SwiGLU FFN) on 8 Trainium2 NeuronCores.

Strategy: expert-parallel with H-split load balancing. The gate/top-k
routing runs on host (bit-exact with the reference: jax on CPU). Experts
are paired big-with-small by token count; each pair owns two cores, and
each core computes ONE H-half (1536 of 3072) of BOTH experts in the
pair. Per-core work is then (maxbig + maxsmall)/2 token-FFNs instead of
maxbig, which shaves ~3% off the tensor-engine roofline vs plain
expert-per-core. The host sums the two half-H partial outputs (fp32)
and scatter-adds into the full [B,S,D] output.

Problem dims (hardcoded): B=4, S=2048, D=1024, E=8, TOP_K=2, H=3072.

Perf notes (from NTFF traces):
- The PE matmul stream runs at the bf16 roofline (~N/2.4GHz per MM)
  once data is resident; an early version lost ~26us waiting for the
  gpsimd/SWDGE queue (boots at ~12us, ~76GB/s) to deliver chunk-0
  tokens, while the sync HWDGE queue moved weights at ~400GB/s.
- ALL input DMA rides the sync HWDGE queue, ordered by first use:
  chunk-0 tokens (8 k-slices so the first ht-group can start as they
  land), w13 in per-ht blocks (host-prepacked so each block is one
  contiguous 4KB-per-partition transfer), wts, next tokens, w2, then
  the second phase's weights. Stage A consumes w13 ht-blocks as they
  arrive (3.4us compute per block vs ~1.4us DMA) so the PE starts
  ~11us in and never starves.
- Output (yg) DMA uses the scalar HWDGE queue so it never head-of-line
  blocks input prefetches on the sync queue.
- Chunks are always >=256 wide: at N<256 the 128-column LDWEIGHTS
  stream (~107ns, 1.2GHz) outpaces the matmul stream and the PE gets
  LDW-paced. This is also why token counts are padded to 128-multiples
  instead of compiled exactly.

SBUF budget per partition (bytes): w13 A+B 96K, w2 A+B 48K, xg 16K,
g 12K, sig+sil 6K, ot 6K -> ~184K of 208K.
PSUM: ps1 x2 + ps3 x2 (stage A) + pso x4 (stage B) = 8 banks.
"""

import sys
import types

if "/opt/trn_rl_repo" not in sys.path:
    sys.path.insert(0, "/opt/trn_rl_repo")

import numpy as np
import ml_dtypes


def _ensure_axon_hooks_shim():
    """bass_utils imports antenv.axon_hooks when BASS_TRACE is set; this
    image's antenv lacks it. Provide a no-op shim so tracing degrades
    gracefully instead of crashing (a real hook may overwrite it)."""
    try:
        import antenv.axon_hooks  # noqa: F401
        return
    except ImportError:
        pass
    try:
        import antenv
    except ImportError:
        return
    mod = types.ModuleType("antenv.axon_hooks")
    mod._hook = None
    mod.set_axon_ntff_profile_hook = lambda h: setattr(mod, "_hook", h)
    mod.get_axon_ntff_profile_hook = lambda: mod._hook
    sys.modules["antenv.axon_hooks"] = mod
    antenv.axon_hooks = mod


_ensure_axon_hooks_shim()

B, S, D = 4, 2048, 1024
E = 8
TOP_K = 2
H = 3 * D
T = B * S
KD = D // 128     # 8  k-tiles over D
HH = H // 2       # 1536: H-half per core
NHH = HH // 128   # 12 h-tiles per half
ND = D // 512     # 2  512-wide output column tiles
BLK = 2 * KD * 128  # free-dim extent of one w13 ht-block (w1|w3 x 8 k-tiles)

BF16 = ml_dtypes.bfloat16

_nc_cache: dict = {}


def _chunk_list(C):
    """Token chunks (PSUM bank free dim <= 512). All chunk STARTS are
    128-aligned (wts indexing) and all chunks are >=231 wide (at N much
    below ~256 the 128-column LDWEIGHTS stream, ~107ns @1.2GHz, outpaces
    the matmul stream and the PE gets LDW-paced). Only the last chunk
    may be a non-multiple of 128: C itself need not be padded - matmul
    free dims are arbitrary, so the tail is exact (no zero-token cols).
    Chunk 0 must be 512 so stage A compute per w13 ht-block (3.4us)
    outpaces the block DMA (~1.4us)."""
    chunks = []
    rem = C
    while rem > 512 + 384:
        chunks.append(512)
        rem -= 512
    if rem <= 512:
        chunks.append(rem)
    else:
        # split so the first part is a multiple of 128 and both >= 231
        first = min(512, ((rem - 231) // 128) * 128)
        chunks.extend([first, rem - first])
    return chunks


def build_pair_ffn(CA: int, CB: int):
    """Bass program for one core: SwiGLU FFN over one H-half of two
    experts (A: CA tokens, B: CB tokens).

    Inputs (all host-prepacked, per core):
      xgkA  [128, KD*CA]       bf16 : xgkA[p, k*CA+c] = xA[token c, k*128+p]
      w13bA [128, NHH*BLK]     bf16 : per-ht blocks of this core's H-half:
                                      w13bA[p, ((ht*2+s)*KD+k)*128+c]
                                        = w{1,3}A[k*128+p, (half*NHH+ht)*128+c]
      w2A   [HH, D]            bf16 : this core's H-half rows of w2A
      wtsA  [128, CA/128]      f32  : combine weight of token n*128+p at [p,n]
      (same for B)
    Outputs:
      ygA [CA, D], ygB [CB, D] f32 : partial (H-half) expert outputs,
                                     wts * (silu(xg@w1h) * (xg@w3h)) @ w2h
    """
    import concourse.bacc as bacc
    import concourse.tile as tile
    import concourse.mybir as mybir

    fp32 = mybir.dt.float32
    bf16 = mybir.dt.bfloat16

    nc = bacc.Bacc("TRN2", target_bir_lowering=False, debug=False, num_devices=8)

    phases = []
    for tag, C in (("A", CA), ("B", CB)):
        CP = ((C + 127) // 128) * 128   # host arrays padded to 128 tokens
        phases.append({
            "tag": tag,
            "C": C,
            "CP": CP,
            "chunks": _chunk_list(C),
            "xgk": nc.dram_tensor(f"xgk{tag}", [128, KD * CP], bf16, kind="ExternalInput"),
            "w13b": nc.dram_tensor(f"w13b{tag}", [128, NHH * BLK], bf16, kind="ExternalInput"),
            "w2": nc.dram_tensor(f"w2{tag}", [HH, D], bf16, kind="ExternalInput"),
            "wts": nc.dram_tensor(f"wts{tag}", [128, CP // 128], fp32, kind="ExternalInput"),
            "yg": nc.dram_tensor(f"yg{tag}", [CP, D], fp32, kind="ExternalOutput"),
        })

    with tile.TileContext(nc) as tc:
        with (
            tc.tile_pool(name="wres", bufs=1) as wres,
            tc.tile_pool(name="xgp", bufs=2) as xgp,
            tc.tile_pool(name="gp", bufs=1) as gp,
            tc.tile_pool(name="tmp", bufs=3) as tmp,
            tc.tile_pool(name="outp", bufs=3) as outp,
            tc.tile_pool(name="psA", bufs=2, space="PSUM") as psA,
            tc.tile_pool(name="psB", bufs=4, space="PSUM") as psB,
        ):
            def load_xg_chunk(ph, c0, NC):
                # one dma_start per chunk ([128, KD, NC] 3D AP): a single
                # ~0.62us sync-engine dispatch; splitting into per-k DMAs
                # was measured WORSE (8 serialized dispatches delay w13b0)
                xgk_3d = ph["xgk"].ap().rearrange("p (k c) -> p k c", k=KD)
                xt = xgp.tile([128, KD * NC], bf16, tag="xg")
                nc.sync.dma_start(
                    xt[:].rearrange("p (k c) -> p k c", k=KD),
                    xgk_3d[:, :, c0:c0 + NC],
                )
                return [xt[:, k * NC:(k + 1) * NC] for k in range(KD)]

            def load_weights(ph):
                tag = ph["tag"]
                w13_sb = []
                for ht in range(NHH):
                    t1 = wres.tile([128, BLK], bf16, tag=f"w13{tag}_{ht}")
                    nc.sync.dma_start(
                        t1[:], ph["w13b"].ap()[:, ht * BLK:(ht + 1) * BLK]
                    )
                    w13_sb.append(t1)
                wts_sb = wres.tile([128, ph["CP"] // 128], fp32, tag=f"wts{tag}")
                nc.sync.dma_start(wts_sb[:], ph["wts"].ap())
                return w13_sb, wts_sb

            def load_w2(ph):
                tag = ph["tag"]
                w2_sb = []
                for ht in range(NHH):
                    t2 = wres.tile([128, D], bf16, tag=f"w2{tag}_{ht}")
                    nc.sync.dma_start(
                        t2[:], ph["w2"].ap()[ht * 128:(ht + 1) * 128, :]
                    )
                    w2_sb.append(t2)
                return w2_sb

            # Sync HWDGE queue FIFO, in consumption order: phase-A chunk-0
            # tokens, phase-A w13 blocks + wts, chunk-1 tokens, phase-A w2,
            # then ALL phase-B weights (consumed from ~270us; they land by
            # ~80us), with remaining token chunks prefetched from the loop.
            pA, pB = phases
            pA["xg0"] = load_xg_chunk(pA, 0, pA["chunks"][0])
            pA["w13_sb"], pA["wts_sb"] = load_weights(pA)
            offsA = [sum(pA["chunks"][:i]) for i in range(len(pA["chunks"]))]
            pA["prefetched"] = {}
            if len(pA["chunks"]) > 1:
                pA["prefetched"][1] = load_xg_chunk(pA, offsA[1], pA["chunks"][1])
            pA["w2_sb"] = load_w2(pA)
            pB["w13_sb"], pB["wts_sb"] = load_weights(pB)
            pB["w2_sb"] = load_w2(pB)
            pB["xg0"] = None
            pB["prefetched"] = {}

            for pi, ph in enumerate(phases):
                chunks = ph["chunks"]
                offs = [sum(chunks[:i]) for i in range(len(chunks))]
                w13_sb, w2_sb, wts_sb = ph["w13_sb"], ph["w2_sb"], ph["wts_sb"]
                yg = ph["yg"]
                prefetched = ph["prefetched"]
                c0 = 0
                for ch, NC in enumerate(chunks):
                    NT = (NC + 127) // 128
                    xg_sb = ph["xg0"] if ch == 0 else prefetched.pop(ch)
                    if xg_sb is None:   # phase B chunk 0: prefetched below
                        xg_sb = prefetched.pop(0)
                    # prefetch one chunk ahead; at this phase's last chunk,
                    # prefetch the next phase's chunk 0
                    if ch + 1 < len(chunks):
                        if ch + 1 not in prefetched:
                            prefetched[ch + 1] = load_xg_chunk(
                                ph, offs[ch + 1], chunks[ch + 1]
                            )
                    elif pi + 1 < len(phases):
                        nxt = phases[pi + 1]
                        nxt["prefetched"][0] = load_xg_chunk(
                            nxt, 0, nxt["chunks"][0]
                        )

                    # stage A: g[h, tok] = silu(y1) * y3 for this H-half
                    g_tiles = []
                    for ht in range(NHH):
                        # first chunk of phase A: stage B is idle, so borrow
                        # psB's banks for extra in-flight groups while the
                        # w13 blocks are still arriving
                        pool = psB if (pi == 0 and ch == 0 and ht % 2 == 1) else psA
                        ps1 = pool.tile([128, NC], fp32, tag="ps1" if pool is psA else "pso")
                        ps3 = pool.tile([128, NC], fp32, tag="ps3" if pool is psA else "pso")
                        wt = w13_sb[ht]
                        for k in range(KD):
                            nc.tensor.matmul(
                                ps1[:],
                                wt[:, k * 128:(k + 1) * 128],
                                xg_sb[k],
                                start=(k == 0),
                                stop=(k == KD - 1),
                            )
                        for k in range(KD):
                            nc.tensor.matmul(
                                ps3[:],
                                wt[:, (KD + k) * 128:(KD + k + 1) * 128],
                                xg_sb[k],
                                start=(k == 0),
                                stop=(k == KD - 1),
                            )
                        sig = tmp.tile([128, NC], fp32, tag="sig")
                        nc.scalar.activation(
                            sig[:], ps1[:], mybir.ActivationFunctionType.Sigmoid
                        )
                        sil = tmp.tile([128, NC], fp32, tag="sil")
                        nc.vector.tensor_mul(sil[:], sig[:], ps1[:])
                        gt = gp.tile([128, NC], bf16, tag=f"g_{ht}")
                        nc.vector.tensor_mul(gt[:], sil[:], ps3[:])
                        g_tiles.append(gt)

                    # stage B: yg[tok, d] = wts[tok] * (g.T @ w2half)
                    for tt in range(NT):
                        gtile_idx = c0 // 128 + tt
                        pt = min(128, NC - tt * 128)   # exact tail width
                        ot = outp.tile([128, D], fp32, tag="ot")
                        for dh in range(ND):
                            pso = psB.tile([128, 512], fp32, tag="pso")
                            for ht in range(NHH):
                                nc.tensor.matmul(
                                    pso[:pt, :],
                                    g_tiles[ht][:, tt * 128:tt * 128 + pt],
                                    w2_sb[ht][:, dh * 512:(dh + 1) * 512],
                                    start=(ht == 0),
                                    stop=(ht == NHH - 1),
                                )
                            nc.vector.tensor_scalar_mul(
                                ot[:pt, dh * 512:(dh + 1) * 512], pso[:pt, :],
                                wts_sb[:pt, gtile_idx:gtile_idx + 1],
                            )
                        # one contiguous [pt, 1024] store per token-tile.
                        # Phase A outputs ride the scalar HWDGE queue so they
                        # never head-of-line block input prefetches on sync;
                        # phase B outputs ride sync (idle after ~100us), which
                        # halves the per-queue drain at kernel end.
                        oq = nc.scalar if pi == 0 else nc.sync
                        oq.dma_start(
                            yg.ap()[c0 + tt * 128:c0 + tt * 128 + pt, :],
                            ot[:pt, :],
                        )
                    c0 += NC

    nc.compile()
    return nc


def route_host(xf: np.ndarray, gate_w: np.ndarray):
    """Top-2 routing, bit-exact with the reference (jax on CPU)."""
    import jax
    import jax.numpy as jnp

    cpu = jax.devices("cpu")[0]
    with jax.default_device(cpu):
        xj = jax.device_put(xf, cpu)
        gj = jax.device_put(gate_w, cpu)
        probs = jax.nn.softmax(xj @ gj, axis=-1)
        vals, idx = jax.lax.top_k(probs, TOP_K)
        w = vals / jnp.sum(vals, axis=-1, keepdims=True)
    return np.asarray(idx), np.asarray(w)


def prepare_dispatch(x, gate_w):
    """Host routing + per-expert gather lists + big/small pairing."""
    xf = np.ascontiguousarray(np.asarray(x).reshape(T, D), dtype=np.float32)
    gate_w = np.asarray(gate_w, dtype=np.float32)
    idx, w = route_host(xf, gate_w)
    tok_flat = np.repeat(np.arange(T), TOP_K)
    idx_flat = idx.ravel()
    w_flat = w.astype(np.float32).ravel()
    toks = []
    wts_list = []
    for e in range(E):
        sel = idx_flat == e
        toks.append(tok_flat[sel])
        wts_list.append(w_flat[sel])
    counts = np.array([len(t) for t in toks])
    order = np.argsort(-counts, kind="stable")
    # pair largest with smallest: pair i = (order[i], order[E-1-i])
    pairs = [(int(order[i]), int(order[E - 1 - i])) for i in range(E // 2)]
    # exact (unpadded) segment capacities; matmul free dims are arbitrary
    CA = max(256, int(counts[order[0]]))
    CB = max(256, int(counts[order[E // 2]]))
    return xf, toks, wts_list, pairs, CA, CB


def _pack_tokens(xf_bf, toks_e, C):
    """xgk [128, KD*CP]: xgk[p, k*CP+c] = x[token c, k*128+p] (bf16)."""
    CP = ((C + 127) // 128) * 128
    xgT = np.zeros((D, CP), dtype=BF16)
    xgT[:, :len(toks_e)] = xf_bf[toks_e].T
    return np.ascontiguousarray(
        xgT.reshape(KD, 128, CP).transpose(1, 0, 2).reshape(128, -1)
    )


def _pack_wts(wts_e, C):
    CP = ((C + 127) // 128) * 128
    wflat = np.zeros(CP, dtype=np.float32)
    wflat[:len(wts_e)] = wts_e
    return np.ascontiguousarray(wflat.reshape(CP // 128, 128).T)


def _pack_w13_half(w1_e, w3_e, half):
    """Per-ht blocks of one H-half: [128, NHH*BLK], block ht is
    [w1 k-tiles | w3 k-tiles], each k-tile 128 cols contiguous."""
    sl = slice(half * HH, (half + 1) * HH)
    w1b = np.asarray(w1_e[:, sl], dtype=np.float32).astype(BF16)
    w3b = np.asarray(w3_e[:, sl], dtype=np.float32).astype(BF16)
    # [D, HH] -> [k, p, ht, c] -> [p, ht, (s), k, c]
    w1r = w1b.reshape(KD, 128, NHH, 128).transpose(1, 2, 0, 3)
    w3r = w3b.reshape(KD, 128, NHH, 128).transpose(1, 2, 0, 3)
    return np.ascontiguousarray(np.stack([w1r, w3r], axis=2).reshape(128, -1))


def make_in_maps(xf, toks, wts_list, pairs, CA, CB, w1, w2, w3):
    xf_bf = xf.astype(BF16)
    in_maps = [None] * E
    for pi, (ea, eb) in enumerate(pairs):
        xgkA = _pack_tokens(xf_bf, toks[ea], CA)
        xgkB = _pack_tokens(xf_bf, toks[eb], CB)
        wtsA = _pack_wts(wts_list[ea], CA)
        wtsB = _pack_wts(wts_list[eb], CB)
        for half in range(2):
            in_maps[2 * pi + half] = {
                "xgkA": xgkA,
                "xgkB": xgkB,
                "wtsA": wtsA,
                "wtsB": wtsB,
                "w13bA": _pack_w13_half(w1[ea], w3[ea], half),
                "w13bB": _pack_w13_half(w1[eb], w3[eb], half),
                "w2A": np.ascontiguousarray(
                    np.asarray(w2[ea][half * HH:(half + 1) * HH], dtype=np.float32)
                ).astype(BF16),
                "w2B": np.ascontiguousarray(
                    np.asarray(w2[eb][half * HH:(half + 1) * HH], dtype=np.float32)
                ).astype(BF16),
            }
    return in_maps


def combine_outputs(results, toks, pairs):
    out = np.zeros((T, D), dtype=np.float32)
    for pi, (ea, eb) in enumerate(pairs):
        r0, r1 = results[2 * pi], results[2 * pi + 1]
        na, nb = len(toks[ea]), len(toks[eb])
        out[toks[ea]] += np.asarray(r0["ygA"][:na], dtype=np.float32)
        out[toks[ea]] += np.asarray(r1["ygA"][:na], dtype=np.float32)
        out[toks[eb]] += np.asarray(r0["ygB"][:nb], dtype=np.float32)
        out[toks[eb]] += np.asarray(r1["ygB"][:nb], dtype=np.float32)
    return out.reshape(B, S, D)


def run(x, gate_w, w1, w2, w3, **spmd_kwargs):
    """Run the MoE. Returns (output, BassKernelResults)."""
    from concourse import bass_utils

    xf, toks, wts_list, pairs, CA, CB = prepare_dispatch(x, gate_w)
    key = (CA, CB)
    if key not in _nc_cache:
        _nc_cache[key] = build_pair_ffn(CA, CB)
    nc = _nc_cache[key]

    in_maps = make_in_maps(xf, toks, wts_list, pairs, CA, CB, w1, w2, w3)
    res = bass_utils.run_bass_kernel_spmd(
        nc, in_maps, core_ids=list(range(E)), **spmd_kwargs
    )
    out = combine_outputs(res.results, toks, pairs).astype(
        np.asarray(x).dtype, copy=False
    )
    return out, res


def kernel(x, gate_w, w1, w2, w3):
    out, _ = run(x, gate_w, w1, w2, w3)
    return out


# revision 24
# speedup vs baseline: 1.0197x; 1.0006x over previous
"""MoE (top-2 of 8 experts, SwiGLU FFN) on 8 Trainium2 NeuronCores.

Strategy: expert-parallel with H-split load balancing. The gate/top-k
routing runs on host (bit-exact with the reference: jax on CPU). Experts
are paired big-with-small by token count; each pair owns two cores, and
each core computes ONE H-half (1536 of 3072) of BOTH experts in the
pair. Per-core work is then (maxbig + maxsmall)/2 token-FFNs instead of
maxbig, which shaves ~3% off the tensor-engine roofline vs plain
expert-per-core. The host sums the two half-H partial outputs (fp32)
and scatter-adds into the full [B,S,D] output.

Problem dims (hardcoded): B=4, S=2048, D=1024, E=8, TOP_K=2, H=3072.

Perf notes (from NTFF traces):
- The PE matmul stream runs at the bf16 roofline (~N/2.4GHz per MM)
  once data is resident; an early version lost ~26us waiting for the
  gpsimd/SWDGE queue (boots at ~12us, ~76GB/s) to deliver chunk-0
  tokens, while the sync HWDGE queue moved weights at ~400GB/s.
- ALL input DMA rides the sync HWDGE queue, ordered by first use:
  chunk-0 tokens (one 3D-AP dma_start; per-k slices measured WORSE,
  ~0.62us sync-engine time per dispatch), w13 in per-ht blocks
  (host-prepacked so each block is one contiguous 4KB-per-partition
  transfer), wts, next tokens, w2, then the second phase's weights.
  Stage A consumes w13 ht-blocks as they arrive (3.4us compute per
  block vs ~1.4us DMA) so the PE starts ~14us in and never starves.
- Phase-A output (yg) DMA uses the scalar HWDGE queue so it never
  head-of-line blocks input prefetches on sync; phase-B output rides
  sync (idle by then), splitting the end-of-kernel drain.
- Chunks are >=231 wide: at small N the 128-column LDWEIGHTS stream
  (~107ns, 1.2GHz) outpaces the matmul stream and the PE gets
  LDW-paced. Token counts are exact (matmul free dims are arbitrary);
  only chunk STARTS must be 128-aligned for wts indexing.

SBUF budget per partition (bytes): w13 A+B 96K, w2 A+B 48K, xg 16K,
g 12K, sig+sil 6K, ot 6K -> ~184K of 208K.
PSUM: ps1 x2 + ps3 x2 (stage A) + pso x4 (stage B) = 8 banks.
"""

import sys
import types

if "/opt/trn_rl_repo" not in sys.path:
    sys.path.insert(0, "/opt/trn_rl_repo")

import numpy as np
import ml_dtypes


def _ensure_axon_hooks_shim():
    """bass_utils imports antenv.axon_hooks when BASS_TRACE is set; this
    image's antenv lacks it. Provide a no-op shim so tracing degrades
    gracefully instead of crashing (a real hook may overwrite it)."""
    try:
        import antenv.axon_hooks  # noqa: F401
        return
    except ImportError:
        pass
    try:
        import antenv
    except ImportError:
        return
    mod = types.ModuleType("antenv.axon_hooks")
    mod._hook = None
    mod.set_axon_ntff_profile_hook = lambda h: setattr(mod, "_hook", h)
    mod.get_axon_ntff_profile_hook = lambda: mod._hook
    sys.modules["antenv.axon_hooks"] = mod
    antenv.axon_hooks = mod


_ensure_axon_hooks_shim()

B, S, D = 4, 2048, 1024
E = 8
TOP_K = 2
H = 3 * D
T = B * S
KD = D // 128     # 8  k-tiles over D
HH = H // 2       # 1536: H-half per core
NHH = HH // 128   # 12 h-tiles per half
ND = D // 512     # 2  512-wide output column tiles
BLK = 2 * KD * 128  # free-dim extent of one w13 ht-block (w1|w3 x 8 k-tiles)

BF16 = ml_dtypes.bfloat16

_nc_cache: dict = {}


def _chunk_list(C):
    """Token chunks (PSUM bank free dim <= 512). All chunk STARTS are
    128-aligned (wts indexing) and all chunks are >=231 wide (at N much
    below ~256 the 128-column LDWEIGHTS stream, ~107ns @1.2GHz, outpaces
    the matmul stream and the PE gets LDW-paced). Only the last chunk
    may be a non-multiple of 128: C itself need not be padded - matmul
    free dims are arbitrary, so the tail is exact (no zero-token cols).
    Chunk 0 must be 512 so stage A compute per w13 ht-block (3.4us)
    outpaces the block DMA (~1.4us)."""
    chunks = []
    rem = C
    while rem > 512 + 384:
        chunks.append(512)
        rem -= 512
    if rem <= 512:
        chunks.append(rem)
    else:
        # split so the first part is a multiple of 128 and both >= 231
        first = min(512, ((rem - 231) // 128) * 128)
        chunks.extend([first, rem - first])
    return chunks


def build_pair_ffn(CA: int, CB: int):
    """Bass program for one core: SwiGLU FFN over one H-half of two
    experts (A: CA tokens, B: CB tokens).

    Inputs (all host-prepacked, per core):
      xgkA  [128, KD*CA]       bf16 : xgkA[p, k*CA+c] = xA[token c, k*128+p]
      w13bA [128, NHH*BLK]     bf16 : per-ht blocks of this core's H-half:
                                      w13bA[p, ((ht*2+s)*KD+k)*128+c]
                                        = w{1,3}A[k*128+p, (half*NHH+ht)*128+c]
      w2A   [HH, D]            bf16 : this core's H-half rows of w2A
      wtsA  [128, CA/128]      f32  : combine weight of token n*128+p at [p,n]
      (same for B)
    Outputs:
      ygA [CA, D], ygB [CB, D] f32 : partial (H-half) expert outputs,
                                     wts * (silu(xg@w1h) * (xg@w3h)) @ w2h
    """
    import concourse.bacc as bacc
    import concourse.tile as tile
    import concourse.mybir as mybir

    fp32 = mybir.dt.float32
    bf16 = mybir.dt.bfloat16

    nc = bacc.Bacc("TRN2", target_bir_lowering=False, debug=False, num_devices=8)

    phases = []
    for tag, C in (("A", CA), ("B", CB)):
        CP = ((C + 127) // 128) * 128   # host arrays padded to 128 tokens
        phases.append({
            "tag": tag,
            "C": C,
            "CP": CP,
            "chunks": _chunk_list(C),
            "xgk": nc.dram_tensor(f"xgk{tag}", [128, KD * CP], bf16, kind="ExternalInput"),
            "w13b": nc.dram_tensor(f"w13b{tag}", [128, NHH * BLK], bf16, kind="ExternalInput"),
            "w2": nc.dram_tensor(f"w2{tag}", [HH, D], bf16, kind="ExternalInput"),
            "wts": nc.dram_tensor(f"wts{tag}", [128, CP // 128], fp32, kind="ExternalInput"),
            "yg": nc.dram_tensor(f"yg{tag}", [CP, D], fp32, kind="ExternalOutput"),
        })

    with tile.TileContext(nc) as tc:
        with (
            tc.tile_pool(name="wres", bufs=1) as wres,
            tc.tile_pool(name="xgp", bufs=2) as xgp,
            tc.tile_pool(name="gp", bufs=1) as gp,
            tc.tile_pool(name="tmp", bufs=3) as tmp,
            tc.tile_pool(name="outp", bufs=3) as outp,
            tc.tile_pool(name="psA", bufs=2, space="PSUM") as psA,
            tc.tile_pool(name="psB", bufs=4, space="PSUM") as psB,
        ):
            def load_xg_chunk(ph, c0, NC):
                # one dma_start per chunk ([128, KD, NC] 3D AP): a single
                # ~0.62us sync-engine dispatch; splitting into per-k DMAs
                # was measured WORSE (8 serialized dispatches delay w13b0)
                xgk_3d = ph["xgk"].ap().rearrange("p (k c) -> p k c", k=KD)
                xt = xgp.tile([128, KD * NC], bf16, tag="xg")
                nc.sync.dma_start(
                    xt[:].rearrange("p (k c) -> p k c", k=KD),
                    xgk_3d[:, :, c0:c0 + NC],
                )
                return [xt[:, k * NC:(k + 1) * NC] for k in range(KD)]

            def load_weights(ph):
                tag = ph["tag"]
                w13_sb = []
                for ht in range(NHH):
                    t1 = wres.tile([128, BLK], bf16, tag=f"w13{tag}_{ht}")
                    nc.sync.dma_start(
                        t1[:], ph["w13b"].ap()[:, ht * BLK:(ht + 1) * BLK]
                    )
                    w13_sb.append(t1)
                wts_sb = wres.tile([128, ph["CP"] // 128], fp32, tag=f"wts{tag}")
                nc.sync.dma_start(wts_sb[:], ph["wts"].ap())
                return w13_sb, wts_sb

            def load_w2(ph):
                tag = ph["tag"]
                w2_sb = []
                for ht in range(NHH):
                    t2 = wres.tile([128, D], bf16, tag=f"w2{tag}_{ht}")
                    nc.sync.dma_start(
                        t2[:], ph["w2"].ap()[ht * 128:(ht + 1) * 128, :]
                    )
                    w2_sb.append(t2)
                return w2_sb

            # Sync HWDGE queue FIFO, in consumption order: phase-A chunk-0
            # tokens, phase-A w13 blocks + wts, chunk-1 tokens, phase-A w2,
            # then ALL phase-B weights (consumed from ~270us; they land by
            # ~80us), with remaining token chunks prefetched from the loop.
            pA, pB = phases
            pA["xg0"] = load_xg_chunk(pA, 0, pA["chunks"][0])
            pA["w13_sb"], pA["wts_sb"] = load_weights(pA)
            offsA = [sum(pA["chunks"][:i]) for i in range(len(pA["chunks"]))]
            pA["prefetched"] = {}
            if len(pA["chunks"]) > 1:
                pA["prefetched"][1] = load_xg_chunk(pA, offsA[1], pA["chunks"][1])
            pA["w2_sb"] = load_w2(pA)
            pB["w13_sb"], pB["wts_sb"] = load_weights(pB)
            pB["w2_sb"] = load_w2(pB)
            pB["xg0"] = None
            pB["prefetched"] = {}

            for pi, ph in enumerate(phases):
                chunks = ph["chunks"]
                offs = [sum(chunks[:i]) for i in range(len(chunks))]
                w13_sb, w2_sb, wts_sb = ph["w13_sb"], ph["w2_sb"], ph["wts_sb"]
                yg = ph["yg"]
                prefetched = ph["prefetched"]
                c0 = 0
                for ch, NC in enumerate(chunks):
                    NT = (NC + 127) // 128
                    xg_sb = ph["xg0"] if ch == 0 else prefetched.pop(ch)
                    if xg_sb is None:   # phase B chunk 0: prefetched below
                        xg_sb = prefetched.pop(0)
                    # prefetch one chunk ahead; at this phase's last chunk,
                    # prefetch the next phase's chunk 0
                    if ch + 1 < len(chunks):
                        if ch + 1 not in prefetched:
                            prefetched[ch + 1] = load_xg_chunk(
                                ph, offs[ch + 1], chunks[ch + 1]
                            )
                    elif pi + 1 < len(phases):
                        nxt = phases[pi + 1]
                        nxt["prefetched"][0] = load_xg_chunk(
                            nxt, 0, nxt["chunks"][0]
                        )

                    # stage A: g[h, tok] = silu(y1) * y3 for this H-half
                    g_tiles = []
                    for ht in range(NHH):
                        # first chunk of phase A: stage B is idle, so borrow
                        # psB's banks for extra in-flight groups while the
                        # w13 blocks are still arriving
                        pool = psB if (pi == 0 and ch == 0 and ht % 2 == 1) else psA
                        ps1 = pool.tile([128, NC], fp32, tag="ps1" if pool is psA else "pso")
                        ps3 = pool.tile([128, NC], fp32, tag="ps3" if pool is psA else "pso")
                        wt = w13_sb[ht]
                        for k in range(KD):
                            nc.tensor.matmul(
                                ps1[:],
                                wt[:, k * 128:(k + 1) * 128],
                                xg_sb[k],
                                start=(k == 0),
                                stop=(k == KD - 1),
                            )
                        for k in range(KD):
                            nc.tensor.matmul(
                                ps3[:],
                                wt[:, (KD + k) * 128:(KD + k + 1) * 128],
                                xg_sb[k],
                                start=(k == 0),
                                stop=(k == KD - 1),
                            )
                        sig = tmp.tile([128, NC], fp32, tag="sig")
                        nc.scalar.activation(
                            sig[:], ps1[:], mybir.ActivationFunctionType.Sigmoid
                        )
                        sil = tmp.tile([128, NC], fp32, tag="sil")
                        nc.vector.tensor_mul(sil[:], sig[:], ps1[:])
                        gt = gp.tile([128, NC], bf16, tag=f"g_{ht}")
                        nc.vector.tensor_mul(gt[:], sil[:], ps3[:])
                        g_tiles.append(gt)

                    # stage B: yg[tok, d] = wts[tok] * (g.T @ w2half)
                    for tt in range(NT):
                        gtile_idx = c0 // 128 + tt
                        pt = min(128, NC - tt * 128)   # exact tail width
                        ot = outp.tile([128, D], fp32, tag="ot")
                        for dh in range(ND):
                            pso = psB.tile([128, 512], fp32, tag="pso")
                            for ht in range(NHH):
                                nc.tensor.matmul(
                                    pso[:pt, :],
                                    g_tiles[ht][:, tt * 128:tt * 128 + pt],
                                    w2_sb[ht][:, dh * 512:(dh + 1) * 512],
                                    start=(ht == 0),
                                    stop=(ht == NHH - 1),
                                )
                            nc.vector.tensor_scalar_mul(
                                ot[:pt, dh * 512:(dh + 1) * 512], pso[:pt, :],
                                wts_sb[:pt, gtile_idx:gtile_idx + 1],
                            )
                        # one contiguous [pt, 1024] store per token-tile.
                        # Phase A outputs ride the scalar HWDGE queue so they
                        # never head-of-line block input prefetches on sync;
                        # phase B outputs ride sync (idle after ~100us), which
                        # halves the per-queue drain at kernel end.
                        oq = nc.scalar if pi == 0 else nc.sync
                        oq.dma_start(
                            yg.ap()[c0 + tt * 128:c0 + tt * 128 + pt, :],
                            ot[:pt, :],
                        )
                    c0 += NC

    nc.compile()
    return nc


def route_host(xf: np.ndarray, gate_w: np.ndarray):
    """Top-2 routing, bit-exact with the reference (jax on CPU)."""
    import jax
    import jax.numpy as jnp

    cpu = jax.devices("cpu")[0]
    with jax.default_device(cpu):
        xj = jax.device_put(xf, cpu)
        gj = jax.device_put(gate_w, cpu)
        probs = jax.nn.softmax(xj @ gj, axis=-1)
        vals, idx = jax.lax.top_k(probs, TOP_K)
        w = vals / jnp.sum(vals, axis=-1, keepdims=True)
    return np.asarray(idx), np.asarray(w)


def prepare_dispatch(x, gate_w):
    """Host routing + per-expert gather lists + big/small pairing."""
    xf = np.ascontiguousarray(np.asarray(x).reshape(T, D), dtype=np.float32)
    gate_w = np.asarray(gate_w, dtype=np.float32)
    idx, w = route_host(xf, gate_w)
    tok_flat = np.repeat(np.arange(T), TOP_K)
    idx_flat = idx.ravel()
    w_flat = w.astype(np.float32).ravel()
    toks = []
    wts_list = []
    for e in range(E):
        sel = idx_flat == e
        toks.append(tok_flat[sel])
        wts_list.append(w_flat[sel])
    counts = np.array([len(t) for t in toks])
    order = np.argsort(-counts, kind="stable")
    # pair largest with smallest: pair i = (order[i], order[E-1-i])
    pairs = [(int(order[i]), int(order[E - 1 - i])) for i in range(E // 2)]
    # exact (unpadded) segment capacities; matmul free dims are arbitrary
    CA = max(256, int(counts[order[0]]))
    CB = max(256, int(counts[order[E // 2]]))
    return xf, toks, wts_list, pairs, CA, CB


def _pack_tokens(xf_bf, toks_e, C):
    """xgk [128, KD*CP]: xgk[p, k*CP+c] = x[token c, k*128+p] (bf16)."""
    CP = ((C + 127) // 128) * 128
    xgT = np.zeros((D, CP), dtype=BF16)
    xgT[:, :len(toks_e)] = xf_bf[toks_e].T
    return np.ascontiguousarray(
        xgT.reshape(KD, 128, CP).transpose(1, 0, 2).reshape(128, -1)
    )


def _pack_wts(wts_e, C):
    CP = ((C + 127) // 128) * 128
    wflat = np.zeros(CP, dtype=np.float32)
    wflat[:len(wts_e)] = wts_e
    return np.ascontiguousarray(wflat.reshape(CP // 128, 128).T)


def _pack_w13_half(w1_e, w3_e, half):
    """Per-ht blocks of one H-half: [128, NHH*BLK], block ht is
    [w1 k-tiles | w3 k-tiles], each k-tile 128 cols contiguous."""
    sl = slice(half * HH, (half + 1) * HH)
    w1b = np.asarray(w1_e[:, sl], dtype=np.float32).astype(BF16)
    w3b = np.asarray(w3_e[:, sl], dtype=np.float32).astype(BF16)
    # [D, HH] -> [k, p, ht, c] -> [p, ht, (s), k, c]
    w1r = w1b.reshape(KD, 128, NHH, 128).transpose(1, 2, 0, 3)
    w3r = w3b.reshape(KD, 128, NHH, 128).transpose(1, 2, 0, 3)
    return np.ascontiguousarray(np.stack([w1r, w3r], axis=2).reshape(128, -1))


def make_in_maps(xf, toks, wts_list, pairs, CA, CB, w1, w2, w3):
    xf_bf = xf.astype(BF16)
    in_maps = [None] * E
    for pi, (ea, eb) in enumerate(pairs):
        xgkA = _pack_tokens(xf_bf, toks[ea], CA)
        xgkB = _pack_tokens(xf_bf, toks[eb], CB)
        wtsA = _pack_wts(wts_list[ea], CA)
        wtsB = _pack_wts(wts_list[eb], CB)
        for half in range(2):
            in_maps[2 * pi + half] = {
                "xgkA": xgkA,
                "xgkB": xgkB,
                "wtsA": wtsA,
                "wtsB": wtsB,
                "w13bA": _pack_w13_half(w1[ea], w3[ea], half),
                "w13bB": _pack_w13_half(w1[eb], w3[eb], half),
                "w2A": np.ascontiguousarray(
                    np.asarray(w2[ea][half * HH:(half + 1) * HH], dtype=np.float32)
                ).astype(BF16),
                "w2B": np.ascontiguousarray(
                    np.asarray(w2[eb][half * HH:(half + 1) * HH], dtype=np.float32)
                ).astype(BF16),
            }
    return in_maps


def combine_outputs(results, toks, pairs):
    out = np.zeros((T, D), dtype=np.float32)
    for pi, (ea, eb) in enumerate(pairs):
        r0, r1 = results[2 * pi], results[2 * pi + 1]
        na, nb = len(toks[ea]), len(toks[eb])
        out[toks[ea]] += np.asarray(r0["ygA"][:na], dtype=np.float32)
        out[toks[ea]] += np.asarray(r1["ygA"][:na], dtype=np.float32)
        out[toks[eb]] += np.asarray(r0["ygB"][:nb], dtype=np.float32)
        out[toks[eb]] += np.asarray(r1["ygB"][:nb], dtype=np.float32)
    return out.reshape(B, S, D)


def run(x, gate_w, w1, w2, w3, **spmd_kwargs):
    """Run the MoE. Returns (output, BassKernelResults)."""
    from concourse import bass_utils

    xf, toks, wts_list, pairs, CA, CB = prepare_dispatch(x, gate_w)
    key = (CA, CB)
    if key not in _nc_cache:
        _nc_cache[key] = build_pair_ffn(CA, CB)
    nc = _nc_cache[key]

    in_maps = make_in_maps(xf, toks, wts_list, pairs, CA, CB, w1, w2, w3)
    res = bass_utils.run_bass_kernel_spmd(
        nc, in_maps, core_ids=list(range(E)), **spmd_kwargs
    )
    out = combine_outputs(res.results, toks, pairs).astype(
        np.asarray(x).dtype, copy=False
    )
    return out, res


def kernel(x, gate_w, w1, w2, w3):
    out, _ = run(x, gate_w, w1, w2, w3)
    return out


# revision 27
# speedup vs baseline: 1.0241x; 1.0043x over previous
"""MoE (top-2 of 8 experts, SwiGLU FFN) on 8 Trainium2 NeuronCores.

Strategy: expert-parallel with H-split load balancing. The gate/top-k
routing runs on host (bit-exact with the reference: jax on CPU). Experts
are paired big-with-small by token count; each pair owns two cores, and
each core computes ONE H-half (1536 of 3072) of BOTH experts in the
pair. Per-core work is then (maxbig + maxsmall)/2 token-FFNs instead of
maxbig, which shaves ~3% off the tensor-engine roofline vs plain
expert-per-core. The host sums the two half-H partial outputs (fp32)
and scatter-adds into the full [B,S,D] output.

Problem dims (hardcoded): B=4, S=2048, D=1024, E=8, TOP_K=2, H=3072.

Perf notes (from NTFF traces):
- The PE matmul stream runs at the bf16 roofline (~N/2.4GHz per MM)
  once data is resident; an early version lost ~26us waiting for the
  gpsimd/SWDGE queue (boots at ~12us, ~76GB/s) to deliver chunk-0
  tokens, while the sync HWDGE queue moved weights at ~400GB/s.
- ALL input DMA rides the sync HWDGE queue, ordered by first use:
  chunk-0 tokens (one 3D-AP dma_start; per-k slices measured WORSE,
  ~0.62us sync-engine time per dispatch), w13 in per-ht blocks
  (host-prepacked so each block is one contiguous 4KB-per-partition
  transfer), wts, next tokens, w2, then the second phase's weights.
  Stage A consumes w13 ht-blocks as they arrive (3.4us compute per
  block vs ~1.4us DMA) so the PE starts ~14us in and never starves.
- Phase-A output (yg) DMA uses the scalar HWDGE queue so it never
  head-of-line blocks input prefetches on sync; phase-B output rides
  sync (idle by then), splitting the end-of-kernel drain.
- Chunks are >=231 wide: at small N the 128-column LDWEIGHTS stream
  (~107ns, 1.2GHz) outpaces the matmul stream and the PE gets
  LDW-paced. Token counts are exact (matmul free dims are arbitrary);
  only chunk STARTS must be 128-aligned for wts indexing.

SBUF budget per partition (bytes): w13 A+B 96K, w2 A+B 48K, xg 16K,
g 12K, sig+sil 6K, ot 6K -> ~184K of 208K.
PSUM: ps1 x2 + ps3 x2 (stage A) + pso x4 (stage B) = 8 banks.
"""

import sys
import types

if "/opt/trn_rl_repo" not in sys.path:
    sys.path.insert(0, "/opt/trn_rl_repo")

import numpy as np
import ml_dtypes


def _ensure_axon_hooks_shim():
    """bass_utils imports antenv.axon_hooks when BASS_TRACE is set; this
    image's antenv lacks it. Provide a no-op shim so tracing degrades
    gracefully instead of crashing (a real hook may overwrite it)."""
    try:
        import antenv.axon_hooks  # noqa: F401
        return
    except ImportError:
        pass
    try:
        import antenv
    except ImportError:
        return
    mod = types.ModuleType("antenv.axon_hooks")
    mod._hook = None
    mod.set_axon_ntff_profile_hook = lambda h: setattr(mod, "_hook", h)
    mod.get_axon_ntff_profile_hook = lambda: mod._hook
    sys.modules["antenv.axon_hooks"] = mod
    antenv.axon_hooks = mod


_ensure_axon_hooks_shim()

B, S, D = 4, 2048, 1024
E = 8
TOP_K = 2
H = 3 * D
T = B * S
KD = D // 128     # 8  k-tiles over D
HH = H // 2       # 1536: H-half per core
NHH = HH // 128   # 12 h-tiles per half
ND = D // 512     # 2  512-wide output column tiles
BLK = 2 * KD * 128  # free-dim extent of one w13 ht-block (w1|w3 x 8 k-tiles)

BF16 = ml_dtypes.bfloat16

_nc_cache: dict = {}


def _chunk_list(C):
    """Token chunks (PSUM bank free dim <= 512). All chunk STARTS are
    128-aligned (wts indexing) and all chunks are >=231 wide (at N much
    below ~256 the 128-column LDWEIGHTS stream, ~107ns @1.2GHz, outpaces
    the matmul stream and the PE gets LDW-paced). Only the last chunk
    may be a non-multiple of 128: C itself need not be padded - matmul
    free dims are arbitrary, so the tail is exact (no zero-token cols).
    Chunk 0 must be 512 so stage A compute per w13 ht-block (3.4us)
    outpaces the block DMA (~1.4us)."""
    chunks = []
    rem = C
    while rem > 512 + 384:
        chunks.append(512)
        rem -= 512
    if rem <= 512:
        chunks.append(rem)
    else:
        # split so the first part is a multiple of 128 and both >= 231
        first = min(512, ((rem - 231) // 128) * 128)
        chunks.extend([first, rem - first])
    return chunks


def build_pair_ffn(CA: int, CB: int):
    """Bass program for one core: SwiGLU FFN over one H-half of two
    experts (A: CA tokens, B: CB tokens).

    Inputs (all host-prepacked, per core):
      xgkA  [128, KD*CA]       bf16 : xgkA[p, k*CA+c] = xA[token c, k*128+p]
      w13bA [128, NHH*BLK]     bf16 : per-ht blocks of this core's H-half:
                                      w13bA[p, ((ht*2+s)*KD+k)*128+c]
                                        = w{1,3}A[k*128+p, (half*NHH+ht)*128+c]
      w2A   [HH, D]            bf16 : this core's H-half rows of w2A
      wtsA  [128, CA/128]      f32  : combine weight of token n*128+p at [p,n]
      (same for B)
    Outputs:
      ygA [CA, D], ygB [CB, D] f32 : partial (H-half) expert outputs,
                                     wts * (silu(xg@w1h) * (xg@w3h)) @ w2h
    """
    import concourse.bacc as bacc
    import concourse.tile as tile
    import concourse.mybir as mybir

    fp32 = mybir.dt.float32
    bf16 = mybir.dt.bfloat16

    nc = bacc.Bacc("TRN2", target_bir_lowering=False, debug=False, num_devices=8)

    phases = []
    for tag, C in (("A", CA), ("B", CB)):
        CP = ((C + 127) // 128) * 128   # host arrays padded to 128 tokens
        phases.append({
            "tag": tag,
            "C": C,
            "CP": CP,
            "chunks": _chunk_list(C),
            "xgk": nc.dram_tensor(f"xgk{tag}", [128, KD * CP], bf16, kind="ExternalInput"),
            "w13b": nc.dram_tensor(f"w13b{tag}", [128, NHH * BLK], bf16, kind="ExternalInput"),
            "w2": nc.dram_tensor(f"w2{tag}", [HH, D], bf16, kind="ExternalInput"),
            "wts": nc.dram_tensor(f"wts{tag}", [128, CP // 128], fp32, kind="ExternalInput"),
            "yg": nc.dram_tensor(f"yg{tag}", [CP, D], fp32, kind="ExternalOutput"),
        })

    with tile.TileContext(nc) as tc:
        with (
            tc.tile_pool(name="wres", bufs=1) as wres,
            tc.tile_pool(name="xgp", bufs=2) as xgp,
            tc.tile_pool(name="gp", bufs=1) as gp,
            tc.tile_pool(name="tmp", bufs=3) as tmp,
            tc.tile_pool(name="outp", bufs=3) as outp,
            tc.tile_pool(name="psA", bufs=2, space="PSUM") as psA,
            tc.tile_pool(name="psB", bufs=4, space="PSUM") as psB,
        ):
            # HAM prewarm: the PE clock gate sits at K=4/8 (1.2GHz) until
            # ~3.4us of sustained matmul activity. Real data lands ~13us in;
            # a dozen dummy matmuls on a memset tile (vector engine is idle
            # at ~6.5us) warm the gate to 8/8 (2.4GHz) before the first real
            # MM, saving the ~1.7us cold-clock penalty. Reuses a ps1 PSUM
            # buffer (pool rotation hands it back long before it's needed).
            warm = tmp.tile([128, 512], bf16, tag="warm")
            nc.vector.memset(warm[:], 0.5)
            wps = psA.tile([128, 512], fp32, tag="ps1")
            for i in range(12):
                nc.tensor.matmul(
                    wps[:], warm[:, :128], warm[:],
                    start=(i == 0), stop=(i == 11),
                )

            def load_xg_chunk(ph, c0, NC):
                # one dma_start per chunk ([128, KD, NC] 3D AP): a single
                # ~0.62us sync-engine dispatch; splitting into per-k DMAs
                # was measured WORSE (8 serialized dispatches delay w13b0)
                xgk_3d = ph["xgk"].ap().rearrange("p (k c) -> p k c", k=KD)
                xt = xgp.tile([128, KD * NC], bf16, tag="xg")
                nc.sync.dma_start(
                    xt[:].rearrange("p (k c) -> p k c", k=KD),
                    xgk_3d[:, :, c0:c0 + NC],
                )
                return [xt[:, k * NC:(k + 1) * NC] for k in range(KD)]

            def load_weights(ph):
                # separate w1/w3 tiles per ht-block (two DMAs over adjacent
                # DRAM halves): the first ps1 group only waits on the w1
                # half, shaving ~0.26MB off the startup critical path
                tag = ph["tag"]
                w13_sb = []
                for ht in range(NHH):
                    t1 = wres.tile([128, BLK // 2], bf16, tag=f"w1{tag}_{ht}")
                    nc.sync.dma_start(
                        t1[:], ph["w13b"].ap()[:, ht * BLK:ht * BLK + BLK // 2]
                    )
                    t3 = wres.tile([128, BLK // 2], bf16, tag=f"w3{tag}_{ht}")
                    nc.sync.dma_start(
                        t3[:], ph["w13b"].ap()[:, ht * BLK + BLK // 2:(ht + 1) * BLK]
                    )
                    w13_sb.append((t1, t3))
                wts_sb = wres.tile([128, ph["CP"] // 128], fp32, tag=f"wts{tag}")
                nc.sync.dma_start(wts_sb[:], ph["wts"].ap())
                return w13_sb, wts_sb

            def load_w2(ph):
                tag = ph["tag"]
                w2_sb = []
                for ht in range(NHH):
                    t2 = wres.tile([128, D], bf16, tag=f"w2{tag}_{ht}")
                    nc.sync.dma_start(
                        t2[:], ph["w2"].ap()[ht * 128:(ht + 1) * 128, :]
                    )
                    w2_sb.append(t2)
                return w2_sb

            # Sync HWDGE queue FIFO, in consumption order: phase-A chunk-0
            # tokens, phase-A w13 blocks + wts, chunk-1 tokens, phase-A w2,
            # then ALL phase-B weights (consumed from ~270us; they land by
            # ~80us), with remaining token chunks prefetched from the loop.
            pA, pB = phases
            pA["xg0"] = load_xg_chunk(pA, 0, pA["chunks"][0])
            pA["w13_sb"], pA["wts_sb"] = load_weights(pA)
            offsA = [sum(pA["chunks"][:i]) for i in range(len(pA["chunks"]))]
            pA["prefetched"] = {}
            if len(pA["chunks"]) > 1:
                pA["prefetched"][1] = load_xg_chunk(pA, offsA[1], pA["chunks"][1])
            pA["w2_sb"] = load_w2(pA)
            pB["w13_sb"], pB["wts_sb"] = load_weights(pB)
            pB["w2_sb"] = load_w2(pB)
            pB["xg0"] = None
            pB["prefetched"] = {}

            for pi, ph in enumerate(phases):
                chunks = ph["chunks"]
                offs = [sum(chunks[:i]) for i in range(len(chunks))]
                w13_sb, w2_sb, wts_sb = ph["w13_sb"], ph["w2_sb"], ph["wts_sb"]
                yg = ph["yg"]
                prefetched = ph["prefetched"]
                c0 = 0
                for ch, NC in enumerate(chunks):
                    NT = (NC + 127) // 128
                    xg_sb = ph["xg0"] if ch == 0 else prefetched.pop(ch)
                    if xg_sb is None:   # phase B chunk 0: prefetched below
                        xg_sb = prefetched.pop(0)
                    # prefetch one chunk ahead; at this phase's last chunk,
                    # prefetch the next phase's chunk 0
                    if ch + 1 < len(chunks):
                        if ch + 1 not in prefetched:
                            prefetched[ch + 1] = load_xg_chunk(
                                ph, offs[ch + 1], chunks[ch + 1]
                            )
                    elif pi + 1 < len(phases):
                        nxt = phases[pi + 1]
                        nxt["prefetched"][0] = load_xg_chunk(
                            nxt, 0, nxt["chunks"][0]
                        )

                    # stage A: g[h, tok] = silu(y1) * y3 for this H-half
                    g_tiles = []
                    for ht in range(NHH):
                        # first chunk of phase A: stage B is idle, so borrow
                        # psB's banks for extra in-flight groups while the
                        # w13 blocks are still arriving
                        pool = psB if (pi == 0 and ch == 0 and ht % 2 == 1) else psA
                        ps1 = pool.tile([128, NC], fp32, tag="ps1" if pool is psA else "pso")
                        ps3 = pool.tile([128, NC], fp32, tag="ps3" if pool is psA else "pso")
                        wt1, wt3 = w13_sb[ht]
                        for k in range(KD):
                            nc.tensor.matmul(
                                ps1[:],
                                wt1[:, k * 128:(k + 1) * 128],
                                xg_sb[k],
                                start=(k == 0),
                                stop=(k == KD - 1),
                            )
                        for k in range(KD):
                            nc.tensor.matmul(
                                ps3[:],
                                wt3[:, k * 128:(k + 1) * 128],
                                xg_sb[k],
                                start=(k == 0),
                                stop=(k == KD - 1),
                            )
                        sig = tmp.tile([128, NC], fp32, tag="sig")
                        nc.scalar.activation(
                            sig[:], ps1[:], mybir.ActivationFunctionType.Sigmoid
                        )
                        sil = tmp.tile([128, NC], fp32, tag="sil")
                        nc.vector.tensor_mul(sil[:], sig[:], ps1[:])
                        gt = gp.tile([128, NC], bf16, tag=f"g_{ht}")
                        nc.vector.tensor_mul(gt[:], sil[:], ps3[:])
                        g_tiles.append(gt)

                    # stage B: yg[tok, d] = wts[tok] * (g.T @ w2half)
                    for tt in range(NT):
                        gtile_idx = c0 // 128 + tt
                        pt = min(128, NC - tt * 128)   # exact tail width
                        ot = outp.tile([128, D], fp32, tag="ot")
                        for dh in range(ND):
                            pso = psB.tile([128, 512], fp32, tag="pso")
                            for ht in range(NHH):
                                nc.tensor.matmul(
                                    pso[:pt, :],
                                    g_tiles[ht][:, tt * 128:tt * 128 + pt],
                                    w2_sb[ht][:, dh * 512:(dh + 1) * 512],
                                    start=(ht == 0),
                                    stop=(ht == NHH - 1),
                                )
                            nc.vector.tensor_scalar_mul(
                                ot[:pt, dh * 512:(dh + 1) * 512], pso[:pt, :],
                                wts_sb[:pt, gtile_idx:gtile_idx + 1],
                            )
                        # one contiguous [pt, 1024] store per token-tile.
                        # Phase A outputs ride the scalar HWDGE queue so they
                        # never head-of-line block input prefetches on sync;
                        # phase B outputs ride sync (idle after ~100us), which
                        # halves the per-queue drain at kernel end.
                        oq = nc.scalar if pi == 0 else nc.sync
                        oq.dma_start(
                            yg.ap()[c0 + tt * 128:c0 + tt * 128 + pt, :],
                            ot[:pt, :],
                        )
                    c0 += NC

    nc.compile()
    return nc


def route_host(xf: np.ndarray, gate_w: np.ndarray):
    """Top-2 routing, bit-exact with the reference (jax on CPU)."""
    import jax
    import jax.numpy as jnp

    cpu = jax.devices("cpu")[0]
    with jax.default_device(cpu):
        xj = jax.device_put(xf, cpu)
        gj = jax.device_put(gate_w, cpu)
        probs = jax.nn.softmax(xj @ gj, axis=-1)
        vals, idx = jax.lax.top_k(probs, TOP_K)
        w = vals / jnp.sum(vals, axis=-1, keepdims=True)
    return np.asarray(idx), np.asarray(w)


def prepare_dispatch(x, gate_w):
    """Host routing + per-expert gather lists + big/small pairing."""
    xf = np.ascontiguousarray(np.asarray(x).reshape(T, D), dtype=np.float32)
    gate_w = np.asarray(gate_w, dtype=np.float32)
    idx, w = route_host(xf, gate_w)
    tok_flat = np.repeat(np.arange(T), TOP_K)
    idx_flat = idx.ravel()
    w_flat = w.astype(np.float32).ravel()
    toks = []
    wts_list = []
    for e in range(E):
        sel = idx_flat == e
        toks.append(tok_flat[sel])
        wts_list.append(w_flat[sel])
    counts = np.array([len(t) for t in toks])
    order = np.argsort(-counts, kind="stable")
    # pair largest with smallest: pair i = (order[i], order[E-1-i])
    pairs = [(int(order[i]), int(order[E - 1 - i])) for i in range(E // 2)]
    # exact (unpadded) segment capacities; matmul free dims are arbitrary
    CA = max(256, int(counts[order[0]]))
    CB = max(256, int(counts[order[E // 2]]))
    return xf, toks, wts_list, pairs, CA, CB


def _pack_tokens(xf_bf, toks_e, C):
    """xgk [128, KD*CP]: xgk[p, k*CP+c] = x[token c, k*128+p] (bf16)."""
    CP = ((C + 127) // 128) * 128
    xgT = np.zeros((D, CP), dtype=BF16)
    xgT[:, :len(toks_e)] = xf_bf[toks_e].T
    return np.ascontiguousarray(
        xgT.reshape(KD, 128, CP).transpose(1, 0, 2).reshape(128, -1)
    )


def _pack_wts(wts_e, C):
    CP = ((C + 127) // 128) * 128
    wflat = np.zeros(CP, dtype=np.float32)
    wflat[:len(wts_e)] = wts_e
    return np.ascontiguousarray(wflat.reshape(CP // 128, 128).T)


def _pack_w13_half(w1_e, w3_e, half):
    """Per-ht blocks of one H-half: [128, NHH*BLK], block ht is
    [w1 k-tiles | w3 k-tiles], each k-tile 128 cols contiguous."""
    sl = slice(half * HH, (half + 1) * HH)
    w1b = np.asarray(w1_e[:, sl], dtype=np.float32).astype(BF16)
    w3b = np.asarray(w3_e[:, sl], dtype=np.float32).astype(BF16)
    # [D, HH] -> [k, p, ht, c] -> [p, ht, (s), k, c]
    w1r = w1b.reshape(KD, 128, NHH, 128).transpose(1, 2, 0, 3)
    w3r = w3b.reshape(KD, 128, NHH, 128).transpose(1, 2, 0, 3)
    return np.ascontiguousarray(np.stack([w1r, w3r], axis=2).reshape(128, -1))


def make_in_maps(xf, toks, wts_list, pairs, CA, CB, w1, w2, w3):
    xf_bf = xf.astype(BF16)
    in_maps = [None] * E
    for pi, (ea, eb) in enumerate(pairs):
        xgkA = _pack_tokens(xf_bf, toks[ea], CA)
        xgkB = _pack_tokens(xf_bf, toks[eb], CB)
        wtsA = _pack_wts(wts_list[ea], CA)
        wtsB = _pack_wts(wts_list[eb], CB)
        for half in range(2):
            in_maps[2 * pi + half] = {
                "xgkA": xgkA,
                "xgkB": xgkB,
                "wtsA": wtsA,
                "wtsB": wtsB,
                "w13bA": _pack_w13_half(w1[ea], w3[ea], half),
                "w13bB": _pack_w13_half(w1[eb], w3[eb], half),
                "w2A": np.ascontiguousarray(
                    np.asarray(w2[ea][half * HH:(half + 1) * HH], dtype=np.float32)
                ).astype(BF16),
                "w2B": np.ascontiguousarray(
                    np.asarray(w2[eb][half * HH:(half + 1) * HH], dtype=np.float32)
                ).astype(BF16),
            }
    return in_maps


def combine_outputs(results, toks, pairs):
    out = np.zeros((T, D), dtype=np.float32)
    for pi, (ea, eb) in enumerate(pairs):
        r0, r1 = results[2 * pi], results[2 * pi + 1]
        na, nb = len(toks[ea]), len(toks[eb])
        out[toks[ea]] += np.asarray(r0["ygA"][:na], dtype=np.float32)
        out[toks[ea]] += np.asarray(r1["ygA"][:na], dtype=np.float32)
        out[toks[eb]] += np.asarray(r0["ygB"][:nb], dtype=np.float32)
        out[toks[eb]] += np.asarray(r1["ygB"][:nb], dtype=np.float32)
    return out.reshape(B, S, D)


def run(x, gate_w, w1, w2, w3, **spmd_kwargs):
    """Run the MoE. Returns (output, BassKernelResults)."""
    from concourse import bass_utils

    xf, toks, wts_list, pairs, CA, CB = prepare_dispatch(x, gate_w)
    key = (CA, CB)
    if key not in _nc_cache:
        _nc_cache[key] = build_pair_ffn(CA, CB)
    nc = _nc_cache[key]

    in_maps = make_in_maps(xf, toks, wts_list, pairs, CA, CB, w1, w2, w3)
    res = bass_utils.run_bass_kernel_spmd(
        nc, in_maps, core_ids=list(range(E)), **spmd_kwargs
    )
    out = combine_outputs(res.results, toks, pairs).astype(
        np.asarray(x).dtype, copy=False
    )
    return out, res


def kernel(x, gate_w, w1, w2, w3):
    out, _ = run(x, gate_w, w1, w2, w3)
    return out


# revision 28
# speedup vs baseline: 1.0272x; 1.0030x over previous
"""MoE (top-2 of 8 experts, SwiGLU FFN) on 8 Trainium2 NeuronCores.

Strategy: expert-parallel with H-split load balancing. The gate/top-k
routing runs on host (bit-exact with the reference: jax on CPU). Experts
are paired big-with-small by token count; each pair owns two cores, and
each core computes ONE H-half (1536 of 3072) of BOTH experts in the
pair. Per-core work is then (maxbig + maxsmall)/2 token-FFNs instead of
maxbig, which shaves ~3% off the tensor-engine roofline vs plain
expert-per-core. The host sums the two half-H partial outputs (fp32)
and scatter-adds into the full [B,S,D] output.

Problem dims (hardcoded): B=4, S=2048, D=1024, E=8, TOP_K=2, H=3072.

Perf notes (from NTFF traces):
- The PE matmul stream runs at the bf16 roofline (~N/2.4GHz per MM)
  once data is resident; an early version lost ~26us waiting for the
  gpsimd/SWDGE queue (boots at ~12us, ~76GB/s) to deliver chunk-0
  tokens, while the sync HWDGE queue moved weights at ~400GB/s.
- ALL input DMA rides the sync HWDGE queue, ordered by first use:
  chunk-0 tokens (one 3D-AP dma_start; per-k slices measured WORSE,
  ~0.62us sync-engine time per dispatch), w13 in per-ht blocks
  (host-prepacked so each block is one contiguous 4KB-per-partition
  transfer), wts, next tokens, w2, then the second phase's weights.
  Stage A consumes w13 ht-blocks as they arrive (3.4us compute per
  block vs ~1.4us DMA) so the PE starts ~14us in and never starves.
- Phase-A output (yg) DMA uses the scalar HWDGE queue so it never
  head-of-line blocks input prefetches on sync; phase-B output rides
  sync (idle by then), splitting the end-of-kernel drain.
- Chunks are >=231 wide: at small N the 128-column LDWEIGHTS stream
  (~107ns, 1.2GHz) outpaces the matmul stream and the PE gets
  LDW-paced. Token counts are exact (matmul free dims are arbitrary);
  only chunk STARTS must be 128-aligned for wts indexing.

SBUF budget per partition (bytes): w13 A+B 96K, w2 A+B 48K, xg 16K,
g 12K, sig+sil 6K, ot 6K -> ~184K of 208K.
PSUM: ps1 x2 + ps3 x2 (stage A) + pso x4 (stage B) = 8 banks.
"""

import sys
import types

if "/opt/trn_rl_repo" not in sys.path:
    sys.path.insert(0, "/opt/trn_rl_repo")

import numpy as np
import ml_dtypes


def _ensure_axon_hooks_shim():
    """bass_utils imports antenv.axon_hooks when BASS_TRACE is set; this
    image's antenv lacks it. Provide a no-op shim so tracing degrades
    gracefully instead of crashing (a real hook may overwrite it)."""
    try:
        import antenv.axon_hooks  # noqa: F401
        return
    except ImportError:
        pass
    try:
        import antenv
    except ImportError:
        return
    mod = types.ModuleType("antenv.axon_hooks")
    mod._hook = None
    mod.set_axon_ntff_profile_hook = lambda h: setattr(mod, "_hook", h)
    mod.get_axon_ntff_profile_hook = lambda: mod._hook
    sys.modules["antenv.axon_hooks"] = mod
    antenv.axon_hooks = mod


_ensure_axon_hooks_shim()

B, S, D = 4, 2048, 1024
E = 8
TOP_K = 2
H = 3 * D
T = B * S
KD = D // 128     # 8  k-tiles over D
HH = H // 2       # 1536: H-half per core
NHH = HH // 128   # 12 h-tiles per half
ND = D // 512     # 2  512-wide output column tiles
BLK = 2 * KD * 128  # free-dim extent of one w13 ht-block (w1|w3 x 8 k-tiles)

BF16 = ml_dtypes.bfloat16

_nc_cache: dict = {}


def _chunk_list(C, first=None):
    """Token chunks (PSUM bank free dim <= 512). All chunk STARTS are
    128-aligned (wts indexing) and all chunks are >=231 wide (at N much
    below ~256 the 128-column LDWEIGHTS stream, ~107ns @1.2GHz, outpaces
    the matmul stream and the PE gets LDW-paced). Only the last chunk
    may be a non-multiple of 128: C itself need not be padded - matmul
    free dims are arbitrary, so the tail is exact (no zero-token cols).
    `first` sets the leading chunk width: 256 for phase A (smaller xg0
    -> first real MM ~1.4us earlier; stage A still outpaces the block
    DMA at 256: 1.7us compute vs ~0.7us delivery)."""
    chunks = []
    rem = C
    if first is not None and C > first + 231:
        chunks.append(first)
        rem -= first
    while rem > 512 + 384:
        chunks.append(512)
        rem -= 512
    if rem <= 512:
        chunks.append(rem)
    else:
        # split so the first part is a multiple of 128 and both >= 231
        first = min(512, ((rem - 231) // 128) * 128)
        chunks.extend([first, rem - first])
    return chunks


def build_pair_ffn(CA: int, CB: int):
    """Bass program for one core: SwiGLU FFN over one H-half of two
    experts (A: CA tokens, B: CB tokens).

    Inputs (all host-prepacked, per core):
      xgkA  [128, KD*CA]       bf16 : xgkA[p, k*CA+c] = xA[token c, k*128+p]
      w13bA [128, NHH*BLK]     bf16 : per-ht blocks of this core's H-half:
                                      w13bA[p, ((ht*2+s)*KD+k)*128+c]
                                        = w{1,3}A[k*128+p, (half*NHH+ht)*128+c]
      w2A   [HH, D]            bf16 : this core's H-half rows of w2A
      wtsA  [128, CA/128]      f32  : combine weight of token n*128+p at [p,n]
      (same for B)
    Outputs:
      ygA [CA, D], ygB [CB, D] f32 : partial (H-half) expert outputs,
                                     wts * (silu(xg@w1h) * (xg@w3h)) @ w2h
    """
    import concourse.bacc as bacc
    import concourse.tile as tile
    import concourse.mybir as mybir

    fp32 = mybir.dt.float32
    bf16 = mybir.dt.bfloat16

    nc = bacc.Bacc("TRN2", target_bir_lowering=False, debug=False, num_devices=8)

    phases = []
    for tag, C in (("A", CA), ("B", CB)):
        CP = ((C + 127) // 128) * 128   # host arrays padded to 128 tokens
        phases.append({
            "tag": tag,
            "C": C,
            "CP": CP,
            "chunks": _chunk_list(C, first=256 if tag == "A" else None),
            "xgk": nc.dram_tensor(f"xgk{tag}", [128, KD * CP], bf16, kind="ExternalInput"),
            "w13b": nc.dram_tensor(f"w13b{tag}", [128, NHH * BLK], bf16, kind="ExternalInput"),
            "w2": nc.dram_tensor(f"w2{tag}", [HH, D], bf16, kind="ExternalInput"),
            "wts": nc.dram_tensor(f"wts{tag}", [128, CP // 128], fp32, kind="ExternalInput"),
            "yg": nc.dram_tensor(f"yg{tag}", [CP, D], fp32, kind="ExternalOutput"),
        })

    with tile.TileContext(nc) as tc:
        with (
            tc.tile_pool(name="wres", bufs=1) as wres,
            tc.tile_pool(name="xgp", bufs=2) as xgp,
            tc.tile_pool(name="gp", bufs=1) as gp,
            tc.tile_pool(name="tmp", bufs=3) as tmp,
            tc.tile_pool(name="outp", bufs=3) as outp,
            tc.tile_pool(name="psA", bufs=2, space="PSUM") as psA,
            tc.tile_pool(name="psB", bufs=4, space="PSUM") as psB,
        ):
            # HAM prewarm: the PE clock gate sits at K=4/8 (1.2GHz) until
            # ~3.4us of sustained matmul activity. Real data lands ~13us in;
            # a dozen dummy matmuls on a memset tile (vector engine is idle
            # at ~6.5us) warm the gate to 8/8 (2.4GHz) before the first real
            # MM, saving the ~1.7us cold-clock penalty. Reuses a ps1 PSUM
            # buffer (pool rotation hands it back long before it's needed).
            warm = tmp.tile([128, 512], bf16, tag="warm")
            nc.vector.memset(warm[:], 0.5)
            wps = psA.tile([128, 512], fp32, tag="ps1")
            for i in range(12):
                nc.tensor.matmul(
                    wps[:], warm[:, :128], warm[:],
                    start=(i == 0), stop=(i == 11),
                )

            def load_xg_chunk(ph, c0, NC):
                # one dma_start per chunk ([128, KD, NC] 3D AP): a single
                # ~0.62us sync-engine dispatch; splitting into per-k DMAs
                # was measured WORSE (8 serialized dispatches delay w13b0)
                xgk_3d = ph["xgk"].ap().rearrange("p (k c) -> p k c", k=KD)
                xt = xgp.tile([128, KD * NC], bf16, tag="xg")
                nc.sync.dma_start(
                    xt[:].rearrange("p (k c) -> p k c", k=KD),
                    xgk_3d[:, :, c0:c0 + NC],
                )
                return [xt[:, k * NC:(k + 1) * NC] for k in range(KD)]

            def load_weights(ph):
                # separate w1/w3 tiles per ht-block (two DMAs over adjacent
                # DRAM halves): the first ps1 group only waits on the w1
                # half, shaving ~0.26MB off the startup critical path
                tag = ph["tag"]
                w13_sb = []
                for ht in range(NHH):
                    t1 = wres.tile([128, BLK // 2], bf16, tag=f"w1{tag}_{ht}")
                    nc.sync.dma_start(
                        t1[:], ph["w13b"].ap()[:, ht * BLK:ht * BLK + BLK // 2]
                    )
                    t3 = wres.tile([128, BLK // 2], bf16, tag=f"w3{tag}_{ht}")
                    nc.sync.dma_start(
                        t3[:], ph["w13b"].ap()[:, ht * BLK + BLK // 2:(ht + 1) * BLK]
                    )
                    w13_sb.append((t1, t3))
                wts_sb = wres.tile([128, ph["CP"] // 128], fp32, tag=f"wts{tag}")
                nc.sync.dma_start(wts_sb[:], ph["wts"].ap())
                return w13_sb, wts_sb

            def load_w2(ph):
                tag = ph["tag"]
                w2_sb = []
                for ht in range(NHH):
                    t2 = wres.tile([128, D], bf16, tag=f"w2{tag}_{ht}")
                    nc.sync.dma_start(
                        t2[:], ph["w2"].ap()[ht * 128:(ht + 1) * 128, :]
                    )
                    w2_sb.append(t2)
                return w2_sb

            # Sync HWDGE queue FIFO, in consumption order: phase-A chunk-0
            # tokens, phase-A w13 blocks + wts, chunk-1 tokens, phase-A w2,
            # then ALL phase-B weights (consumed from ~270us; they land by
            # ~80us), with remaining token chunks prefetched from the loop.
            pA, pB = phases
            pA["xg0"] = load_xg_chunk(pA, 0, pA["chunks"][0])
            pA["w13_sb"], pA["wts_sb"] = load_weights(pA)
            offsA = [sum(pA["chunks"][:i]) for i in range(len(pA["chunks"]))]
            pA["prefetched"] = {}
            if len(pA["chunks"]) > 1:
                pA["prefetched"][1] = load_xg_chunk(pA, offsA[1], pA["chunks"][1])
            pA["w2_sb"] = load_w2(pA)
            pB["w13_sb"], pB["wts_sb"] = load_weights(pB)
            pB["w2_sb"] = load_w2(pB)
            pB["xg0"] = None
            pB["prefetched"] = {}

            for pi, ph in enumerate(phases):
                chunks = ph["chunks"]
                offs = [sum(chunks[:i]) for i in range(len(chunks))]
                w13_sb, w2_sb, wts_sb = ph["w13_sb"], ph["w2_sb"], ph["wts_sb"]
                yg = ph["yg"]
                prefetched = ph["prefetched"]
                c0 = 0
                for ch, NC in enumerate(chunks):
                    NT = (NC + 127) // 128
                    xg_sb = ph["xg0"] if ch == 0 else prefetched.pop(ch)
                    if xg_sb is None:   # phase B chunk 0: prefetched below
                        xg_sb = prefetched.pop(0)
                    # prefetch one chunk ahead; at this phase's last chunk,
                    # prefetch the next phase's chunk 0
                    if ch + 1 < len(chunks):
                        if ch + 1 not in prefetched:
                            prefetched[ch + 1] = load_xg_chunk(
                                ph, offs[ch + 1], chunks[ch + 1]
                            )
                    elif pi + 1 < len(phases):
                        nxt = phases[pi + 1]
                        nxt["prefetched"][0] = load_xg_chunk(
                            nxt, 0, nxt["chunks"][0]
                        )

                    # stage A: g[h, tok] = silu(y1) * y3 for this H-half
                    g_tiles = []
                    for ht in range(NHH):
                        # first chunk of phase A: stage B is idle, so borrow
                        # psB's banks for extra in-flight groups while the
                        # w13 blocks are still arriving
                        pool = psB if (pi == 0 and ch == 0 and ht % 2 == 1) else psA
                        ps1 = pool.tile([128, NC], fp32, tag="ps1" if pool is psA else "pso")
                        ps3 = pool.tile([128, NC], fp32, tag="ps3" if pool is psA else "pso")
                        wt1, wt3 = w13_sb[ht]
                        for k in range(KD):
                            nc.tensor.matmul(
                                ps1[:],
                                wt1[:, k * 128:(k + 1) * 128],
                                xg_sb[k],
                                start=(k == 0),
                                stop=(k == KD - 1),
                            )
                        for k in range(KD):
                            nc.tensor.matmul(
                                ps3[:],
                                wt3[:, k * 128:(k + 1) * 128],
                                xg_sb[k],
                                start=(k == 0),
                                stop=(k == KD - 1),
                            )
                        sig = tmp.tile([128, NC], fp32, tag="sig")
                        nc.scalar.activation(
                            sig[:], ps1[:], mybir.ActivationFunctionType.Sigmoid
                        )
                        sil = tmp.tile([128, NC], fp32, tag="sil")
                        nc.vector.tensor_mul(sil[:], sig[:], ps1[:])
                        gt = gp.tile([128, NC], bf16, tag=f"g_{ht}")
                        nc.vector.tensor_mul(gt[:], sil[:], ps3[:])
                        g_tiles.append(gt)

                    # stage B: yg[tok, d] = wts[tok] * (g.T @ w2half)
                    for tt in range(NT):
                        gtile_idx = c0 // 128 + tt
                        pt = min(128, NC - tt * 128)   # exact tail width
                        # very last token-tile of the kernel: store per-dh
                        # so the first half transfers while the second
                        # half's matmuls run (shrinks the end drain)
                        last_out = (
                            pi == len(phases) - 1
                            and ch == len(chunks) - 1
                            and tt == NT - 1
                        )
                        ot = outp.tile([128, D], fp32, tag="ot")
                        for dh in range(ND):
                            pso = psB.tile([128, 512], fp32, tag="pso")
                            for ht in range(NHH):
                                nc.tensor.matmul(
                                    pso[:pt, :],
                                    g_tiles[ht][:, tt * 128:tt * 128 + pt],
                                    w2_sb[ht][:, dh * 512:(dh + 1) * 512],
                                    start=(ht == 0),
                                    stop=(ht == NHH - 1),
                                )
                            nc.vector.tensor_scalar_mul(
                                ot[:pt, dh * 512:(dh + 1) * 512], pso[:pt, :],
                                wts_sb[:pt, gtile_idx:gtile_idx + 1],
                            )
                            if last_out:
                                oq = nc.scalar if pi == 0 else nc.sync
                                oq.dma_start(
                                    yg.ap()[
                                        c0 + tt * 128:c0 + tt * 128 + pt,
                                        dh * 512:(dh + 1) * 512,
                                    ],
                                    ot[:pt, dh * 512:(dh + 1) * 512],
                                )
                        # one contiguous [pt, 1024] store per token-tile.
                        # Phase A outputs ride the scalar HWDGE queue so they
                        # never head-of-line block input prefetches on sync;
                        # phase B outputs ride sync (idle after ~100us), which
                        # halves the per-queue drain at kernel end.
                        if not last_out:
                            oq = nc.scalar if pi == 0 else nc.sync
                            oq.dma_start(
                                yg.ap()[c0 + tt * 128:c0 + tt * 128 + pt, :],
                                ot[:pt, :],
                            )
                    c0 += NC

    nc.compile()
    return nc


def route_host(xf: np.ndarray, gate_w: np.ndarray):
    """Top-2 routing, bit-exact with the reference (jax on CPU)."""
    import jax
    import jax.numpy as jnp

    cpu = jax.devices("cpu")[0]
    with jax.default_device(cpu):
        xj = jax.device_put(xf, cpu)
        gj = jax.device_put(gate_w, cpu)
        probs = jax.nn.softmax(xj @ gj, axis=-1)
        vals, idx = jax.lax.top_k(probs, TOP_K)
        w = vals / jnp.sum(vals, axis=-1, keepdims=True)
    return np.asarray(idx), np.asarray(w)


def prepare_dispatch(x, gate_w):
    """Host routing + per-expert gather lists + big/small pairing."""
    xf = np.ascontiguousarray(np.asarray(x).reshape(T, D), dtype=np.float32)
    gate_w = np.asarray(gate_w, dtype=np.float32)
    idx, w = route_host(xf, gate_w)
    tok_flat = np.repeat(np.arange(T), TOP_K)
    idx_flat = idx.ravel()
    w_flat = w.astype(np.float32).ravel()
    toks = []
    wts_list = []
    for e in range(E):
        sel = idx_flat == e
        toks.append(tok_flat[sel])
        wts_list.append(w_flat[sel])
    counts = np.array([len(t) for t in toks])
    order = np.argsort(-counts, kind="stable")
    # pair largest with smallest: pair i = (order[i], order[E-1-i])
    pairs = [(int(order[i]), int(order[E - 1 - i])) for i in range(E // 2)]
    # exact (unpadded) segment capacities; matmul free dims are arbitrary
    CA = max(256, int(counts[order[0]]))
    CB = max(256, int(counts[order[E // 2]]))
    return xf, toks, wts_list, pairs, CA, CB


def _pack_tokens(xf_bf, toks_e, C):
    """xgk [128, KD*CP]: xgk[p, k*CP+c] = x[token c, k*128+p] (bf16)."""
    CP = ((C + 127) // 128) * 128
    xgT = np.zeros((D, CP), dtype=BF16)
    xgT[:, :len(toks_e)] = xf_bf[toks_e].T
    return np.ascontiguousarray(
        xgT.reshape(KD, 128, CP).transpose(1, 0, 2).reshape(128, -1)
    )


def _pack_wts(wts_e, C):
    CP = ((C + 127) // 128) * 128
    wflat = np.zeros(CP, dtype=np.float32)
    wflat[:len(wts_e)] = wts_e
    return np.ascontiguousarray(wflat.reshape(CP // 128, 128).T)


def _pack_w13_half(w1_e, w3_e, half):
    """Per-ht blocks of one H-half: [128, NHH*BLK], block ht is
    [w1 k-tiles | w3 k-tiles], each k-tile 128 cols contiguous."""
    sl = slice(half * HH, (half + 1) * HH)
    w1b = np.asarray(w1_e[:, sl], dtype=np.float32).astype(BF16)
    w3b = np.asarray(w3_e[:, sl], dtype=np.float32).astype(BF16)
    # [D, HH] -> [k, p, ht, c] -> [p, ht, (s), k, c]
    w1r = w1b.reshape(KD, 128, NHH, 128).transpose(1, 2, 0, 3)
    w3r = w3b.reshape(KD, 128, NHH, 128).transpose(1, 2, 0, 3)
    return np.ascontiguousarray(np.stack([w1r, w3r], axis=2).reshape(128, -1))


def make_in_maps(xf, toks, wts_list, pairs, CA, CB, w1, w2, w3):
    xf_bf = xf.astype(BF16)
    in_maps = [None] * E
    for pi, (ea, eb) in enumerate(pairs):
        xgkA = _pack_tokens(xf_bf, toks[ea], CA)
        xgkB = _pack_tokens(xf_bf, toks[eb], CB)
        wtsA = _pack_wts(wts_list[ea], CA)
        wtsB = _pack_wts(wts_list[eb], CB)
        for half in range(2):
            in_maps[2 * pi + half] = {
                "xgkA": xgkA,
                "xgkB": xgkB,
                "wtsA": wtsA,
                "wtsB": wtsB,
                "w13bA": _pack_w13_half(w1[ea], w3[ea], half),
                "w13bB": _pack_w13_half(w1[eb], w3[eb], half),
                "w2A": np.ascontiguousarray(
                    np.asarray(w2[ea][half * HH:(half + 1) * HH], dtype=np.float32)
                ).astype(BF16),
                "w2B": np.ascontiguousarray(
                    np.asarray(w2[eb][half * HH:(half + 1) * HH], dtype=np.float32)
                ).astype(BF16),
            }
    return in_maps


def combine_outputs(results, toks, pairs):
    out = np.zeros((T, D), dtype=np.float32)
    for pi, (ea, eb) in enumerate(pairs):
        r0, r1 = results[2 * pi], results[2 * pi + 1]
        na, nb = len(toks[ea]), len(toks[eb])
        out[toks[ea]] += np.asarray(r0["ygA"][:na], dtype=np.float32)
        out[toks[ea]] += np.asarray(r1["ygA"][:na], dtype=np.float32)
        out[toks[eb]] += np.asarray(r0["ygB"][:nb], dtype=np.float32)
        out[toks[eb]] += np.asarray(r1["ygB"][:nb], dtype=np.float32)
    return out.reshape(B, S, D)


def run(x, gate_w, w1, w2, w3, **spmd_kwargs):
    """Run the MoE. Returns (output, BassKernelResults)."""
    from concourse import bass_utils

    xf, toks, wts_list, pairs, CA, CB = prepare_dispatch(x, gate_w)
    key = (CA, CB)
    if key not in _nc_cache:
        _nc_cache[key] = build_pair_ffn(CA, CB)
    nc = _nc_cache[key]

    in_maps = make_in_maps(xf, toks, wts_list, pairs, CA, CB, w1, w2, w3)
    res = bass_utils.run_bass_kernel_spmd(
        nc, in_maps, core_ids=list(range(E)), **spmd_kwargs
    )
    out = combine_outputs(res.results, toks, pairs).astype(
        np.asarray(x).dtype, copy=False
    )
    return out, res


def kernel(x, gate_w, w1, w2, w3):
    out, _ = run(x, gate_w, w1, w2, w3)
    return out
